# revision 1
# baseline (speedup 1.0000x reference)
"""Trainium2 Bass kernel for nn_EncoderVidCRN (CRN video QA encoder).

Strategy: pure data parallel over batch B=128 across 8 NeuronCores (16 batch
rows per core). Weights are replicated, cast to bf16 on host, and shipped
pre-transposed into PE-stationary [K, M] layouts with the SBUF partition index
innermost so every device DMA is a plain contiguous [128, ...] copy.

All activations are kept feature-major on device ([d_feature -> partitions,
batch-cols -> free]), so every matmul is psum[M_out_feat, N_cols] =
W_T[K, M].T @ actT[K, N] with no transposes anywhere.

CRN subset means: the reference's rng subset choices are input-independent
(np.random.RandomState(0) at trace time) and replicated here exactly. Means
are computed as unnormalized bf16 subset sums on the vector engine (using a
full-sum minus complement when the complement is smaller), with the 1/|sel|
normalization folded into the g-half of each weight bank on the host.

ELU is composed as relu(x) + min(exp(x), 1) - 1 on ScalarE+VectorE.
"""

import functools
import itertools
import sys

import numpy as np

sys.path.insert(0, "/opt/trn_rl_repo")

import ml_dtypes  # noqa: E402

import concourse.bass as bass  # noqa: E402,F401
import concourse.mybir as mybir  # noqa: E402
import concourse.tile as tile  # noqa: E402
from concourse import bacc  # noqa: E402
from concourse.bass_utils import run_bass_kernel_spmd  # noqa: E402

BF = ml_dtypes.bfloat16
B, C, F, V, D = 128, 8, 16, 2048, 512
NCORES = 8
BS = B // NCORES      # 16 batch rows per core
J = BS * C            # 128 clip-level columns per core
T = F - 4             # 12 retained time slots
JV = BS * T           # 192 video-level columns per core

F32 = mybir.dt.float32
BF16 = mybir.dt.bfloat16
AF = mybir.ActivationFunctionType
OP = mybir.AluOpType

# ---------------------------------------------------------------- subsets


def _subsets():
    """Replicate the reference's rng sequence exactly (trace-time constant)."""
    rng = np.random.RandomState(0)
    out = []
    for n in (F, F - 2, C, C - 2):
        sels = []
        for scale_id in range(1, n - 1):
            scale = n - scale_id
            rels = list(itertools.combinations(range(n), scale))
            idx = rng.choice(len(rels), min(1, len(rels)), replace=False)
            sels.append(list(rels[int(idx[0])]))
        out.append(sels)
    return out


SELS_M, SELS_Q, SELS_VM, SELS_VQ = _subsets()

# bias table layout (f32 [128, 240])
BOFF_A, BOFF_M, BOFF_Q, BOFF_VM, BOFF_G = 0, 4, 8, 12, 16
BOFF_1 = 32            # 14*4
BOFF_2 = 88            # 12*4
BOFF_G2 = 136          # 12*4
BOFF_3 = 184           # 6*4
BOFF_4 = 208           # 4*4
BOFF_G4 = 224          # 4*4
NBIAS = 240

# ---------------------------------------------------------------- device IR


def _gsum(nc, pool, slicer, n_obj, sel, S, shape, tag):
    """Unnormalized bf16 subset sum over object slices.

    slicer(i) -> AP of object i; S = precomputed full sum (or None).
    Uses S - complement when the complement is cheaper.
    """
    in_set = set(sel)
    comp = [i for i in range(n_obj) if i not in in_set]
    use_comp = S is not None and len(comp) + 1 < len(sel)
    if not use_comp and len(sel) == 1:
        return slicer(sel[0])
    out = pool.tile(list(shape), BF16, tag=tag, name=f"gsum_{tag}")
    if use_comp:
        nc.vector.tensor_sub(out, S, slicer(comp[0]))
        for i in comp[1:]:
            nc.vector.tensor_sub(out, out, slicer(i))
    else:
        nc.vector.tensor_add(out, slicer(sel[0]), slicer(sel[1]))
        for i in sel[2:]:
            nc.vector.tensor_add(out, out, slicer(i))
    return out


def _bank_mm(nc, ps_list, wt, g, cond, koff_g, koff_c):
    """psum[m] += Wg[:,m].T @ g + Wc[:,m].T @ cond for the 4 output chunks."""
    for m in range(4):
        ps = ps_list[m]
        for kc in range(4):
            nc.tensor.matmul(ps, wt[:, koff_g + kc, m * 128:(m + 1) * 128],
                             g[:, kc, :], start=(kc == 0), stop=False)
        for kc in range(4):
            nc.tensor.matmul(ps, wt[:, koff_c + kc, m * 128:(m + 1) * 128],
                             cond[:, kc, :], start=False, stop=(kc == 3))


def _elu_group(nc, tpool, ps_list, baps, dsts, cols, gate_list=None,
               neg_gbaps=None, wide_dst=None, view=None):
    """Fused ELU (+ optional sigmoid gate) for four [128, cols] psum slices.

    elu(x) = max(x, min(exp(x), 1) - 1)  (exact since exp(x) - 1 >= x);
    sigmoid(x) = 1/(1 + exp(-x)) so all ACT ops stay in exp_and_others.
    Per-m ops only where the per-m bias forces it; bias-free ops run once at
    4x width.  dsts: per-m dst APs (ungated path); wide_dst: one
    [128, 4, cols]-layout dst AP (gated path); view maps a [128, 4, cols]
    tile onto wide_dst's dim structure.
    """
    t_e = tpool.tile([128, 4, cols], F32, tag="t_exp", name="t_e", bufs=2)
    for m in range(4):
        nc.scalar.activation(t_e[:, m, :], ps_list[m], AF.Exp, bias=baps[m])
    t_m = tpool.tile([128, 4, cols], F32, tag="t_min", name="t_m", bufs=2)
    nc.vector.tensor_scalar(t_m, t_e, 1.0, -1.0, OP.min, OP.add)
    if gate_list is None:
        for m in range(4):
            nc.vector.scalar_tensor_tensor(dsts[m], ps_list[m], baps[m],
                                           t_m[:, m, :], OP.add, OP.max)
        return
    t_z = tpool.tile([128, 4, cols], F32, tag="t_z", name="t_z", bufs=2)
    for m in range(4):
        nc.vector.scalar_tensor_tensor(t_z[:, m, :], ps_list[m], baps[m],
                                       t_m[:, m, :], OP.add, OP.max)
    t_d = tpool.tile([128, 4, cols], F32, tag="t_d", name="t_d", bufs=2)
    for m in range(4):
        nc.scalar.activation(t_d[:, m, :], gate_list[m], AF.Exp,
                             bias=neg_gbaps[m], scale=-1.0)
    nc.vector.tensor_scalar_add(t_d, t_d, 1.0)
    nc.vector.reciprocal(t_d, t_d)
    if view is None:
        view = lambda ap: ap
    nc.vector.tensor_tensor(wide_dst, view(t_z), view(t_d), OP.mult)


def _tree_sum(nc, pool, slicer, n, shape, tag, name):
    """Two-accumulator bf16 sum of n slices (halves the serial DVE chain)."""
    out = pool.tile(list(shape), BF16, tag=tag, name=name)
    half = pool.tile(list(shape), BF16, tag=tag + "_h", name=name + "_h")
    nc.vector.tensor_add(out, slicer(0), slicer(1))
    nc.vector.tensor_add(half, slicer(2), slicer(3))
    for i in range(4, n):
        t = out if i % 2 == 0 else half
        nc.vector.tensor_add(t, t, slicer(i))
    nc.vector.tensor_add(out, out, half)
    return out


@functools.lru_cache(maxsize=2)
def _program(debug=False):
    nc = bacc.Bacc("TRN2", target_bir_lowering=False, debug=False,
                   num_devices=NCORES)

    app_d = nc.dram_tensor("app", [128, 4, 16, 512], BF16, kind="ExternalInput")
    mot_d = nc.dram_tensor("mot", [128, 16, J], BF16, kind="ExternalInput")
    q_d = nc.dram_tensor("q", [128, 4, BS], BF16, kind="ExternalInput")
    wa_d = nc.dram_tensor("wa", [128, 16, 512], BF16, kind="ExternalInput")
    wm_d = nc.dram_tensor("wm", [128, 16, 512], BF16, kind="ExternalInput")
    wq_d = nc.dram_tensor("wq", [128, 4, 512], BF16, kind="ExternalInput")
    wvm_d = nc.dram_tensor("wvm", [128, 4, 512], BF16, kind="ExternalInput")
    wih_d = nc.dram_tensor("wih", [128, 16, 16, 128], BF16, kind="ExternalInput")
    whh_d = nc.dram_tensor("whh", [128, 4, 2048], BF16, kind="ExternalInput")
    w1_d = nc.dram_tensor("w1", [128, 14, 8, 512], BF16, kind="ExternalInput")
    w2_d = nc.dram_tensor("w2", [128, 12, 16, 512], BF16, kind="ExternalInput")
    w3_d = nc.dram_tensor("w3", [128, 6, 8, 512], BF16, kind="ExternalInput")
    w4_d = nc.dram_tensor("w4", [128, 4, 16, 512], BF16, kind="ExternalInput")
    bias_d = nc.dram_tensor("bias", [128, NBIAS], F32, kind="ExternalInput")
    out_d = nc.dram_tensor("out", [128, 4 * 4 * JV], F32, kind="ExternalOutput")
    out_v = out_d.ap().rearrange("p (d s j) -> p d s j", d=4, s=4)
    dbg = {}
    if debug:
        for nm, shape, dt in [("dbg_objsT", [128, 4 * F * J], BF16),
                              ("dbg_objs2T", [128, 4 * 14 * J], BF16),
                              ("dbg_clipT", [128, 4 * C * BS * T], BF16),
                              ("dbg_objs4T", [128, 4 * 6 * JV], BF16),
                              ("dbg_gx", [128, 16 * J], F32),
                              ("dbg_h", [128, 4 * BS], BF16),
                              ("dbg_condm", [128, 4 * J], BF16),
                              ("dbg_qp", [128, 4 * BS], BF16)]:
            dbg[nm] = nc.dram_tensor(nm, shape, dt, kind="ExternalOutput")

    nc._phases = []

    def _mark(name):
        nc._phases.append((name, int(nc.get_next_instruction_name()[2:])))

    with tile.TileContext(nc) as tc:
        # Pools form a strict stack (release order = reverse of allocation).
        perm = tc.alloc_tile_pool(name="perm", bufs=1)
        gpool = tc.alloc_tile_pool(name="gpool", bufs=4)
        tpool = tc.alloc_tile_pool(name="tmp", bufs=4)
        stream = tc.alloc_tile_pool(name="stream", bufs=4)
        p5 = tc.alloc_tile_pool(name="p5", bufs=1)        # clipT
        p4 = tc.alloc_tile_pool(name="p4", bufs=1)        # objs2T
        p3 = tc.alloc_tile_pool(name="p3", bufs=1)        # objsT, condm
        p0 = tc.alloc_tile_pool(name="p0", bufs=1)        # early consts
        pp_early = tc.alloc_tile_pool(name="ps_early", bufs=1, space="PSUM")

        _mark("consts")
        # ---------------- constant loads
        bias = perm.tile([128, NBIAS], F32, name="bias")
        nc.sync.dma_start(bias, bias_d[:])

        def bap(off):
            return bias[:, off:off + 1]

        motT = p0.tile([128, 16, J], BF16, name="motT")
        nc.sync.dma_start(motT, mot_d[:])
        qT = p0.tile([128, 4, BS], BF16, name="qT")
        nc.sync.dma_start(qT, q_d[:])
        wqt = p0.tile([128, 4, 512], BF16, name="wqt")
        nc.sync.dma_start(wqt, wq_d[:])

        _mark("qproj_condm")
        # ---------------- q_proj  [128, 4, BS]
        psq = pp_early.tile([128, 4, BS], F32, tag="psq", name="psq")
        for m in range(4):
            for kc in range(4):
                nc.tensor.matmul(psq[:, m, :], wqt[:, kc, m * 128:(m + 1) * 128],
                                 qT[:, kc, :], start=(kc == 0), stop=(kc == 3))
        qp = perm.tile([128, 4, BS], BF16, name="qp")
        for m in range(4):
            nc.vector.tensor_scalar_add(qp[:, m, :], psq[:, m, :], bap(BOFF_Q + m))

        # ---------------- mot_proj -> cond_m  [128, 4, J]
        wmt_a = stream.tile([128, 8, 512], BF16, tag="crnw8", name="wmt_a")
        nc.sync.dma_start(wmt_a, wm_d[:, 0:8, :])
        wmt_b = stream.tile([128, 8, 512], BF16, tag="crnw8", name="wmt_b")
        nc.sync.dma_start(wmt_b, wm_d[:, 8:16, :])
        pscm = pp_early.tile([128, 4, J], F32, tag="pscm", name="pscm")
        for m in range(4):
            for kc in range(16):
                wmt = wmt_a if kc < 8 else wmt_b
                nc.tensor.matmul(pscm[:, m, :], wmt[:, kc % 8, m * 128:(m + 1) * 128],
                                 motT[:, kc, :], start=(kc == 0), stop=(kc == 15))
        condm = p3.tile([128, 4, J], BF16, name="condm")
        for m in range(4):
            nc.vector.tensor_scalar_add(condm[:, m, :], pscm[:, m, :],
                                        bap(BOFF_M + m))

        # cond_q: q_proj broadcast over clips -> [128, 4, BS, C]
        condq = perm.tile([128, 4, BS, C], BF16, name="condq")
        nc.vector.tensor_copy(condq, qp[:, :, :, None].to_broadcast([128, 4, BS, C]))
        condq_v = condq.rearrange("p d b c -> p d (b c)")
        qvc = perm.tile([128, 4, BS, T], BF16, name="qvc")
        nc.vector.tensor_copy(qvc, qp[:, :, :, None].to_broadcast([128, 4, BS, T]))
        qvc_v = qvc.rearrange("p d b t -> p d (b t)")
        pp_early.release()

        _mark("stageA")
        # ---------------- stage A: app_proj -> objsT [128, 4, F, J]
        p2 = tc.alloc_tile_pool(name="p2", bufs=1)
        apps = tc.alloc_tile_pool(name="apps", bufs=2)
        pp_a = tc.alloc_tile_pool(name="ps_a", bufs=2, space="PSUM")
        wat = p2.tile([128, 16, 512], BF16, name="wat")
        nc.sync.dma_start(wat, wa_d[:])
        objsT = p3.tile([128, 4, F, J], BF16, name="objsT")
        for cc in range(4):
            xc = apps.tile([128, 16, 512], BF16, tag="app", name="xc")
            nc.sync.dma_start(xc, app_d[:, cc, :, :])
            for m in range(4):
                ps_a = pp_a.tile([128, 512], F32, tag="psA", name="ps_a")
                for kc in range(16):
                    nc.tensor.matmul(ps_a, wat[:, kc, m * 128:(m + 1) * 128],
                                     xc[:, kc, :], start=(kc == 0), stop=(kc == 15))
                dst = objsT[:, m, cc * 4:(cc + 1) * 4, :].rearrange("p f j -> p (f j)")
                nc.vector.tensor_scalar_add(dst, ps_a, bap(BOFF_A + m))
        if debug:
            nc.sync.dma_start(dbg["dbg_objsT"][:], objsT.rearrange("p a b c -> p (a b c)"))
        pp_a.release()
        apps.release()
        p2.release()

        _mark("crn_m")
        # ---------------- crn_m: objsT -> objs2T [128, 4, 14, J]
        pp_crn = tc.alloc_tile_pool(name="ps_crn", bufs=2, space="PSUM")
        s_m = _tree_sum(nc, p3, lambda f: objsT[:, :, f, :], F,
                        (128, 4, J), "s_m", "s_m")
        objs2T = p4.tile([128, 4, 14, J], BF16, name="objs2T")
        for si, sel in enumerate(SELS_M):
            w1t = stream.tile([128, 8, 512], BF16, tag="crnw8", name="w1t")
            nc.sync.dma_start(w1t, w1_d[:, si, :, :])
            g = _gsum(nc, gpool, lambda f: objsT[:, :, f, :], F, sel, s_m,
                      (128, 4, J), "g_clip")
            ps = pp_crn.tile([128, 4, J], F32, tag="psM", name="ps_m1", bufs=3)
            _bank_mm(nc, [ps[:, m, :] for m in range(4)], w1t, g, condm, 0, 4)
            _elu_group(nc, tpool, [ps[:, m, :] for m in range(4)],
                       [bap(BOFF_1 + si * 4 + m) for m in range(4)],
                       [objs2T[:, m, si, :] for m in range(4)], J)

        _mark("gatesx")
        # ---------------- LSTM x-gates: gx = W_ih @ motT + (b_ih + b_hh)
        # accumulation groups must be sequential per PSUM bank (start=True
        # clears has_written for the whole bank) -> mi-outer loop.
        wihs = tc.alloc_tile_pool(name="wihs", bufs=3)
        p1 = tc.alloc_tile_pool(name="p1", bufs=1)
        ppx = tc.alloc_tile_pool(name="ps_x", bufs=2, space="PSUM")
        whht = p1.tile([128, 4, 2048], BF16, name="whht")
        nc.sync.dma_start(whht, whh_d[:])
        wvmt = p1.tile([128, 4, 512], BF16, name="wvmt")
        nc.sync.dma_start(wvmt, wvm_d[:])
        gx = p1.tile([128, 16, J], F32, name="gx")
        for mi in range(16):
            wih_t = wihs.tile([128, 16, 128], BF16, tag="wih", name="wih_t")
            nc.sync.dma_start(wih_t, wih_d[:, mi, :, :])
            psx = ppx.tile([128, J], F32, tag="psx", name="psx")
            for kc in range(16):
                nc.tensor.matmul(psx, wih_t[:, kc, :], motT[:, kc, :],
                                 start=(kc == 0), stop=(kc == 15))
            nc.vector.tensor_scalar_add(gx[:, mi, :], psx, bap(BOFF_G + mi))
        ppx.release()
        pp_r = tc.alloc_tile_pool(name="ps_r", bufs=2, space="PSUM")
        # view with the time step as an explicit axis: cols j = b*8 + c
        gxr = gx.rearrange("p m (b c) -> p m c b", c=C)

        _mark("lstm")
        # ---------------- LSTM recurrence (8 steps, h/c are [128, 4, BS])
        # sigmoid(x) = 1/(1+exp(-x)); products become divides so every ACT
        # op stays in the exp_and_others table set.
        h_prev = None
        c_prev = None
        for t in range(C):
            xg = gxr[:, :, t, :]
            if t == 0:
                gates = xg
            else:
                psr = pp_r.tile([128, 16, BS], F32, tag="psr", name="psr")
                for mi in range(16):
                    for kc in range(4):
                        nc.tensor.matmul(psr[:, mi, :],
                                         whht[:, kc, mi * 128:(mi + 1) * 128],
                                         h_prev[:, kc, :],
                                         start=(kc == 0), stop=(kc == 3))
                gates = tpool.tile([128, 16, BS], F32, tag="lstm_g", name="lstm_g")
                nc.vector.tensor_add(gates, psr, xg)
            d_if = tpool.tile([128, 8, BS], F32, tag="dif", name="d_if")
            nc.scalar.activation(d_if, gates[:, 0:8, :], AF.Exp, scale=-1.0)
            nc.vector.tensor_scalar_add(d_if, d_if, 1.0)
            nc.vector.reciprocal(d_if, d_if)
            tan_g = tpool.tile([128, 4, BS], F32, tag="tg", name="tan_g")
            nc.scalar.activation(tan_g, gates[:, 8:12, :], AF.Tanh)
            d_o = tpool.tile([128, 4, BS], F32, tag="do", name="d_o")
            nc.scalar.activation(d_o, gates[:, 12:16, :], AF.Exp, scale=-1.0)
            nc.vector.tensor_scalar_add(d_o, d_o, 1.0)
            nc.vector.reciprocal(d_o, d_o)
            ig = tpool.tile([128, 4, BS], F32, tag="ig", name="ig", bufs=2)
            nc.vector.tensor_tensor(ig, tan_g, d_if[:, 0:4, :], OP.mult)
            if t == 0:
                c_t = ig
            else:
                c_t = tpool.tile([128, 4, BS], F32, tag="c_t", name="c_t", bufs=2)
                fc = tpool.tile([128, 4, BS], F32, tag="fc", name="fc")
                nc.vector.tensor_tensor(fc, c_prev, d_if[:, 4:8, :], OP.mult)
                nc.vector.tensor_add(c_t, fc, ig)
            tan_c = tpool.tile([128, 4, BS], F32, tag="tanc", name="tan_c")
            nc.scalar.activation(tan_c, c_t, AF.Tanh)
            h_t = tpool.tile([128, 4, BS], BF16, tag="h_t", name="h_t", bufs=2)
            nc.vector.tensor_tensor(h_t, tan_c, d_o, OP.mult)
            h_prev, c_prev = h_t, c_t

        # vm_proj -> video cond [128, 4, BS, T]
        psv = pp_r.tile([128, 4, BS], F32, tag="psv", name="psv", bufs=1)
        for m in range(4):
            for kc in range(4):
                nc.tensor.matmul(psv[:, m, :], wvmt[:, kc, m * 128:(m + 1) * 128],
                                 h_prev[:, kc, :], start=(kc == 0), stop=(kc == 3))
        vmp = p1.tile([128, 4, BS], BF16, name="vmp")
        for m in range(4):
            nc.vector.tensor_scalar_add(vmp[:, m, :], psv[:, m, :],
                                        bap(BOFF_VM + m))
        vmc = perm.tile([128, 4, BS, T], BF16, name="vmc")
        nc.vector.tensor_copy(vmc, vmp[:, :, :, None].to_broadcast([128, 4, BS, T]))
        vmc_v = vmc.rearrange("p d b t -> p d (b t)")
        if debug:
            nc.sync.dma_start(dbg["dbg_gx"][:], gx.rearrange("p a b -> p (a b)"))
            nc.sync.dma_start(dbg["dbg_h"][:], h_prev.rearrange("p a b -> p (a b)"))
            nc.sync.dma_start(dbg["dbg_condm"][:], condm.rearrange("p a b -> p (a b)"))
            nc.sync.dma_start(dbg["dbg_qp"][:], qp.rearrange("p a b -> p (a b)"))
        pp_r.release()
        p1.release()
        wihs.release()

        _mark("crn_q")
        # ---------------- crn_q: objs2T -> clipT [128, 4, C, BS, T]
        if debug:
            nc.sync.dma_start(dbg["dbg_objs2T"][:], objs2T.rearrange("p a b c -> p (a b c)"))
        s_2 = _tree_sum(nc, p4, lambda s: objs2T[:, :, s, :], F - 2,
                        (128, 4, J), "s_2", "s_2")
        clipT = p5.tile([128, 4, C, BS, T], BF16, name="clipT")
        for si, sel in enumerate(SELS_Q):
            w2t = stream.tile([128, 8, 512], BF16, tag="crnw8", name="w2t")
            nc.sync.dma_start(w2t, w2_d[:, si, 0:8, :])
            w2g = stream.tile([128, 8, 512], BF16, tag="crnw8", name="w2g")
            nc.sync.dma_start(w2g, w2_d[:, si, 8:16, :])
            g = _gsum(nc, gpool, lambda s: objs2T[:, :, s, :], F - 2, sel, s_2,
                      (128, 4, J), "g_clip")
            ps_m = pp_crn.tile([128, 4, J], F32, tag="psM", name="ps_q1", bufs=3)
            ps_g = pp_crn.tile([128, 4, J], F32, tag="psG", name="ps_q2")
            _bank_mm(nc, [ps_m[:, m, :] for m in range(4)], w2t, g, condq_v, 0, 4)
            _bank_mm(nc, [ps_g[:, m, :] for m in range(4)], w2g, g, condq_v, 0, 4)
            wide = clipT[:, :, :, :, si].rearrange("p d c b -> p d b c")
            _elu_group(nc, tpool, [ps_m[:, m, :] for m in range(4)],
                       [bap(BOFF_2 + si * 4 + m) for m in range(4)], None, J,
                       gate_list=[ps_g[:, m, :] for m in range(4)],
                       neg_gbaps=[bap(BOFF_G2 + si * 4 + m) for m in range(4)],
                       wide_dst=wide,
                       view=lambda ap: ap.rearrange("p d (b c) -> p d b c", c=C))
        if debug:
            nc.sync.dma_start(dbg["dbg_clipT"][:], clipT.rearrange("p a b c d -> p (a b c d)"))
        pp_crn.release()
        p0.release()
        p3.release()
        p4.release()

        _mark("crn_vm")
        # ---------------- crn_vm: clipT -> objs4T [128, 4, 6, JV]
        pp_v = tc.alloc_tile_pool(name="ps_v", bufs=1, space="PSUM")

        def clip_slice(c):
            return clipT[:, :, c, :, :].rearrange("p d b t -> p d (b t)")

        s_3 = _tree_sum(nc, p5, clip_slice, C, (128, 4, JV), "s_3", "s_3")
        objs4T = perm.tile([128, 4, 6, JV], BF16, name="objs4T")
        for si, sel in enumerate(SELS_VM):
            w3t = stream.tile([128, 8, 512], BF16, tag="crnw8", name="w3t")
            nc.sync.dma_start(w3t, w3_d[:, si, :, :])
            g = _gsum(nc, gpool, clip_slice, C, sel, s_3, (128, 4, JV), "g_vid")
            ps0 = pp_v.tile([128, 2, JV], F32, tag="psV0", name="ps_vm0", bufs=2)
            ps1 = pp_v.tile([128, 2, JV], F32, tag="psV1", name="ps_vm1", bufs=2)
            ps_list = [ps0[:, 0, :], ps0[:, 1, :], ps1[:, 0, :], ps1[:, 1, :]]
            _bank_mm(nc, ps_list, w3t, g, vmc_v, 0, 4)
            _elu_group(nc, tpool, ps_list,
                       [bap(BOFF_3 + si * 4 + m) for m in range(4)],
                       [objs4T[:, m, si, :] for m in range(4)], JV)

        _mark("crn_vq")
        # ---------------- crn_vq: objs4T -> out
        if debug:
            nc.sync.dma_start(dbg["dbg_objs4T"][:], objs4T.rearrange("p a b c -> p (a b c)"))

        def o4_slice(s):
            return objs4T[:, :, s, :]

        s_4 = _tree_sum(nc, perm, o4_slice, C - 2, (128, 4, JV), "s_4", "s_4")
        for si, sel in enumerate(SELS_VQ):
            w4t = stream.tile([128, 8, 512], BF16, tag="crnw8", name="w4t")
            nc.sync.dma_start(w4t, w4_d[:, si, 0:8, :])
            w4g = stream.tile([128, 8, 512], BF16, tag="crnw8", name="w4g")
            nc.sync.dma_start(w4g, w4_d[:, si, 8:16, :])
            g = _gsum(nc, gpool, o4_slice, C - 2, sel, s_4, (128, 4, JV), "g_vid")
            ps0 = pp_v.tile([128, 2, JV], F32, tag="psV0", name="ps_vq0", bufs=2)
            ps1 = pp_v.tile([128, 2, JV], F32, tag="psV1", name="ps_vq1", bufs=2)
            pg0 = pp_v.tile([128, 2, JV], F32, tag="psV2", name="ps_vq2")
            pg1 = pp_v.tile([128, 2, JV], F32, tag="psV3", name="ps_vq3")
            ps_list = [ps0[:, 0, :], ps0[:, 1, :], ps1[:, 0, :], ps1[:, 1, :]]
            pg_list = [pg0[:, 0, :], pg0[:, 1, :], pg1[:, 0, :], pg1[:, 1, :]]
            _bank_mm(nc, ps_list, w4t, g, qvc_v, 0, 4)
            _bank_mm(nc, pg_list, w4g, g, qvc_v, 0, 4)
            ot4 = tpool.tile([128, 4, JV], F32, tag="ot", name="ot4", bufs=2)
            _elu_group(nc, tpool, ps_list,
                       [bap(BOFF_4 + si * 4 + m) for m in range(4)], None, JV,
                       gate_list=pg_list,
                       neg_gbaps=[bap(BOFF_G4 + si * 4 + m) for m in range(4)],
                       wide_dst=ot4)
            nc.sync.dma_start(out_v[:, :, si, :], ot4)

        for pool in (pp_v, p5, stream, tpool, gpool, perm):
            pool.release()

    nc.compile()
    return nc


# ---------------------------------------------------------------- host side


def _to_kxm(w_t, kchunks):
    """[K, M] f32 -> [128, kchunks, M] bf16 with partition index innermost."""
    K, M = w_t.shape
    assert K == kchunks * 128
    return np.ascontiguousarray(
        w_t.reshape(kchunks, 128, M).transpose(1, 0, 2)).astype(BF)


def _bank_tensor(Ws, sels, gWs=None):
    """Stack per-scale CRN banks -> [128, S, H*4, 512] bf16.

    Halves order: [Wg/|sel|, Wc] (+ [gWg/|sel|, gWc] when gated); each half is
    the [2D, D] -> [D_in, D_out] transposed stationary operand.
    """
    per = []
    for si, sel in enumerate(sels):
        s_id = si + 1
        halves = [Ws[s_id][:, :D].T / len(sel), Ws[s_id][:, D:].T]
        if gWs is not None:
            halves += [gWs[s_id][:, :D].T / len(sel), gWs[s_id][:, D:].T]
        h = np.stack([np.asarray(x, np.float32) for x in halves])  # [H, 512, 512]
        H = h.shape[0]
        per.append(h.reshape(H, 4, 128, 512).transpose(2, 0, 1, 3)
                   .reshape(128, H * 4, 512))
    return np.ascontiguousarray(np.stack(per, axis=1)).astype(BF)


def _vec_to_pm(v, chunks):
    """[chunks*128] f32 -> [128, chunks] per-partition bias layout."""
    return np.ascontiguousarray(
        np.asarray(v, np.float32).reshape(chunks, 128).T)


@functools.lru_cache(maxsize=1)
def _static_prep_cache():
    return {}


def _prep_weights(inputs):
    w = {}
    w["wa"] = _to_kxm(np.asarray(inputs["Wa"], np.float32).T, 16)
    w["wm"] = _to_kxm(np.asarray(inputs["Wm"], np.float32).T, 16)
    w["wq"] = _to_kxm(np.asarray(inputs["Wq"], np.float32).T, 4)
    w["wvm"] = _to_kxm(np.asarray(inputs["Wvm"], np.float32).T, 4)
    wih = _to_kxm(np.asarray(inputs["W_ih"], np.float32).T, 16)  # [128, kc, 2048]
    w["wih"] = np.ascontiguousarray(
        wih.reshape(128, 16, 16, 128).transpose(0, 2, 1, 3))  # [128, mi, kc, 128]
    w["whh"] = _to_kxm(np.asarray(inputs["W_hh"], np.float32).T, 4)
    w["w1"] = _bank_tensor(np.asarray(inputs["W1"], np.float32), SELS_M)
    w["w2"] = _bank_tensor(np.asarray(inputs["W2"], np.float32), SELS_Q,
                           np.asarray(inputs["gW2"], np.float32))
    w["w3"] = _bank_tensor(np.asarray(inputs["W3"], np.float32), SELS_VM)
    w["w4"] = _bank_tensor(np.asarray(inputs["W4"], np.float32), SELS_VQ,
                           np.asarray(inputs["gW4"], np.float32))

    bias = np.zeros((128, NBIAS), np.float32)
    bias[:, BOFF_A:BOFF_A + 4] = _vec_to_pm(inputs["ba"], 4)
    bias[:, BOFF_M:BOFF_M + 4] = _vec_to_pm(inputs["bm"], 4)
    bias[:, BOFF_Q:BOFF_Q + 4] = _vec_to_pm(inputs["bq"], 4)
    bias[:, BOFF_VM:BOFF_VM + 4] = _vec_to_pm(inputs["bvm"], 4)
    bias[:, BOFF_G:BOFF_G + 16] = _vec_to_pm(
        np.asarray(inputs["b_ih"], np.float32) + np.asarray(inputs["b_hh"], np.float32), 16)
    for si in range(len(SELS_M)):
        bias[:, BOFF_1 + si * 4:BOFF_1 + si * 4 + 4] = _vec_to_pm(inputs["b1"][si + 1], 4)
    for si in range(len(SELS_Q)):
        bias[:, BOFF_2 + si * 4:BOFF_2 + si * 4 + 4] = _vec_to_pm(inputs["b2"][si + 1], 4)
        bias[:, BOFF_G2 + si * 4:BOFF_G2 + si * 4 + 4] = _vec_to_pm(-np.asarray(inputs["gb2"][si + 1], np.float32), 4)
    for si in range(len(SELS_VM)):
        bias[:, BOFF_3 + si * 4:BOFF_3 + si * 4 + 4] = _vec_to_pm(inputs["b3"][si + 1], 4)
    for si in range(len(SELS_VQ)):
        bias[:, BOFF_4 + si * 4:BOFF_4 + si * 4 + 4] = _vec_to_pm(inputs["b4"][si + 1], 4)
        bias[:, BOFF_G4 + si * 4:BOFF_G4 + si * 4 + 4] = _vec_to_pm(-np.asarray(inputs["gb4"][si + 1], np.float32), 4)
    w["bias"] = bias
    return w


def _prep_core_inputs(inputs, core):
    b0 = core * BS
    app = np.asarray(inputs["appearance_video_feat"][b0:b0 + BS], np.float32)
    mot = np.asarray(inputs["motion_video_feat"][b0:b0 + BS], np.float32)
    q = np.asarray(inputs["question_embedding"][b0:b0 + BS], np.float32)
    # app [BS, C, F, V] -> [p, cc, kc, (f4 j)] with 4 f-slots per chunk
    app_t = app.transpose(3, 2, 0, 1).reshape(V, F, J)
    app_t = app_t.reshape(16, 128, F, J).transpose(1, 0, 2, 3)   # [p, kc, f, j]
    app_t = app_t.reshape(128, 16, 4, 4 * J).transpose(0, 2, 1, 3)  # [p, cc, kc, 512]
    # mot [BS, C, V] -> [p, kc, j]
    mot_t = mot.transpose(2, 0, 1).reshape(V, J).reshape(16, 128, J).transpose(1, 0, 2)
    # q [BS, D] -> [p, kc, b]
    q_t = q.T.reshape(4, 128, BS).transpose(1, 0, 2)
    return {
        "app": np.ascontiguousarray(app_t).astype(BF),
        "mot": np.ascontiguousarray(mot_t).astype(BF),
        "q": np.ascontiguousarray(q_t).astype(BF),
    }


def _assemble(results):
    out = np.empty((B, (C - 4) * T, D), np.float32)
    for core in range(NCORES):
        r = results[core]["out"].reshape(128, 4, 4, BS, T)
        # [p, dc, s, b, t] -> [b, s, t, dc, p]
        o = r.transpose(3, 2, 4, 1, 0).reshape(BS, (C - 4) * T, D)
        out[core * BS:(core + 1) * BS] = o
    return out


def build_in_maps(**inputs):
    w = _prep_weights(inputs)
    in_maps = []
    for core in range(NCORES):
        m = dict(w)
        m.update(_prep_core_inputs(inputs, core))
        in_maps.append(m)
    return in_maps


def kernel(**inputs):
    nc = _program(False)
    in_maps = build_in_maps(**inputs)
    res = run_bass_kernel_spmd(nc, in_maps, list(range(NCORES)))
    return _assemble(res.results)


if __name__ == "__main__":
    import reference

    inputs = {k: np.asarray(v) for k, v in reference.setup_inputs().items()}
    out = kernel(**inputs)
    exp = np.asarray(reference.reference(**inputs))
    err = np.abs(out - exp).max() / np.abs(exp).max()
    print("Relative error:", err)



# revision 3
# speedup vs baseline: 1.0069x; 1.0069x over previous
"""Trainium2 Bass kernel for nn_EncoderVidCRN (CRN video QA encoder), fp8 rev.

Data parallel over batch B=128 across 8 NeuronCores (16 rows/core). Mixed
precision chosen from a measured per-tensor error budget (rel gate 2e-2):

 - fp8-e4m3 (DoubleRow matmuls, 2 K-chunks/instruction): appearance proj
   (app, Wa), motion proj (mot, Wm), LSTM x-gates (W_ih), clip CRN banks
   W1/W2/gW2 and their moving operands (objsT, condm, condq, objs2T), video
   CRN-1 bank W3 with moving (clipT, vmc). Weight scale 2048, activation
   scales are fixed powers of two; inverse scales fold into epilogue imms.
 - fp8-e3m4 (plain matmuls, stationary only, scale 64): gW4, W_hh, Wvm.
 - bf16: q path (Wq) and the entire final CRN stage (W4, objs4T, qvc) --
   the only error-sensitive paths (measured: Wq or W4/mov4 in fp8 alone
   would each exceed half the error gate).

Subset means: rng subset choices replicated exactly (trace-time constants).
For the fp8 stages the subset sums run on the PE as extra accumulation
matmuls (direct sum of member objects, or full-sum S minus negated
complement when that needs fewer terms); 1/|sel| folds into the g-half of
each weight bank. The final stage sums on the vector engine in bf16.

ELU: s*elu(x) = max(s*x, min(s*e^x, s) - s) with s*e^x from one Exp
activation (bias ln s). Sigmoids (CRN gates and the LSTM) use the tanh
form sigmoid(x) = (tanh(x/2)+1)/2 so every activation (Exp/Tanh/Copy)
stays in the exp_and_others table set -- no act-table reloads. The LSTM
tracks c2=2c, h2=2h with W_hh/Wvm pre-halved so no extra ops are needed.
Biases are folded per-channel only when any bias input is nonzero (the
general path); the all-zero case (checked host-side) uses wide ops.
"""

import functools
import itertools
import sys

import numpy as np

sys.path.insert(0, "/opt/trn_rl_repo")

import ml_dtypes  # noqa: E402

import concourse.bass as bass  # noqa: E402,F401
import concourse.mybir as mybir  # noqa: E402
import concourse.tile as tile  # noqa: E402
from concourse import bacc  # noqa: E402
from concourse.bass_utils import run_bass_kernel_spmd  # noqa: E402

BF = ml_dtypes.bfloat16
E4 = ml_dtypes.float8_e4m3
E3 = ml_dtypes.float8_e3m4
B, C, F, V, D = 128, 8, 16, 2048, 512
NCORES = 8
BS = B // NCORES      # 16 batch rows per core
J = BS * C            # 128 clip-level columns per core
T = F - 4             # 12 retained time slots
JV = BS * T           # 192 video-level columns per core

F32 = mybir.dt.float32
BF16 = mybir.dt.bfloat16
FP8E4 = mybir.dt.float8e4
FP8E3 = mybir.dt.float8e3
AF = mybir.ActivationFunctionType
OP = mybir.AluOpType
DR = mybir.MatmulPerfMode.DoubleRow

# ---------------------------------------------------------------- scales
SW = 2048.0           # e4m3 weight scale
SW3 = 64.0            # e3m4 weight scale (gW4)
S_APP = 16.0
S_MOT = 16.0
S_OBJ1 = 4.0          # objsT / S1 family
S_CONDM = 8.0
S_QP = 32.0           # condq
S_OBJ2 = 8.0          # objs2T / S2 family
S_OBJ3 = 16.0         # clipT family
S_VMP = 128.0

IMM_OBJS = S_OBJ1 / (SW * S_APP)
IMM_CONDM = S_CONDM / (SW * S_MOT)
INV_GX = 1.0 / (SW * S_MOT)
INV_1 = 1.0 / (SW * S_OBJ1)
IMM_1 = S_OBJ2 * INV_1
INV_2 = 1.0 / (SW * S_OBJ2)
IMM_2 = S_OBJ3 * INV_2
INV_3 = 1.0 / (SW * S_OBJ3)
INV_G4 = 1.0 / SW3
INV_HH = 1.0 / SW3    # whh/wvm ship as e3m4 x 64

# ---------------------------------------------------------------- subsets


def _subsets():
    """Replicate the reference's rng sequence exactly (trace-time constant)."""
    rng = np.random.RandomState(0)
    out = []
    for n in (F, F - 2, C, C - 2):
        sels = []
        for scale_id in range(1, n - 1):
            scale = n - scale_id
            rels = list(itertools.combinations(range(n), scale))
            idx = rng.choice(len(rels), min(1, len(rels)), replace=False)
            sels.append(list(rels[int(idx[0])]))
        out.append(sels)
    return out


SELS_M, SELS_Q, SELS_VM, SELS_VQ = _subsets()

# bias table layout (f32 [128, NBIAS]); constants first, general-path
# per-channel bias columns after.
COL_LN2 = 0     # ln(S_OBJ2)
COL_LN3 = 1     # ln(S_OBJ3 / 2)  (gated: t_z carries s/2)
COL_LNH = 2     # ln(1/2)
BOFF_A, BOFF_M, BOFF_Q, BOFF_VM, BOFF_G = 4, 8, 12, 16, 20
BOFF_1 = 36             # 14*4  (b1 + ln S_OBJ2 for exp; raw*s2 in BOFF_1L)
BOFF_1L = 92            # 14*4  (b1 * S_OBJ2)
BOFF_2 = 148            # 12*4  (b2 + ln(S_OBJ3/2))
BOFF_2L = 196           # 12*4  (b2 * S_OBJ3/2)
BOFF_G2 = 244           # 12*4  (gb2/2, tanh-form gate bias)
BOFF_3 = 292            # 6*4   (b3; exp bias, ln1=0)
BOFF_3L = 316           # 6*4   (b3)
BOFF_4 = 340            # 4*4   (b4 + ln(1/2))
BOFF_4L = 356           # 4*4   (b4/2)
BOFF_G4 = 372           # 4*4   (gb4/2)
NBIAS = 388


def _use_comp(n, sel):
    return (n - len(sel)) + 1 < len(sel)

# ---------------------------------------------------------------- device IR


def _gunits(n, sel, slicer, s_ap, neg_slicer):
    """Moving-operand list for the PE-side subset sum of `sel` over n objects:
    either the member slices, or [S] + negated complement slices."""
    if _use_comp(n, sel):
        in_set = set(sel)
        return [s_ap] + [neg_slicer(i) for i in range(n) if i not in in_set]
    return [slicer(f) for f in sel]


def _dr_group(nc, ps_m, wt_g, wt_c, units, cond_pairs, mslice, cond_first=True):
    """One PSUM accumulation group of DoubleRow matmuls: the cond pairs plus
    the g units. cond_first puts the cond matmuls first (their operands are
    usually ready early, keeping the PE busy across phase barriers); crn_vm
    uses cond_first=False because vmc lands late (after the LSTM chain).
    wt_g/wt_c: [128, 2, 2, 512] stationary halves; units/cond_pairs: lists of
    per-pair moving APs ([128, 2, N])."""
    total = len(units) * 2 + 2
    ops = []
    for u in units:
        for p in range(2):
            ops.append((wt_g[:, p, :, mslice], u[p]))
    cond_ops = [(wt_c[:, p, :, mslice], cond_pairs[p]) for p in range(2)]
    ops = cond_ops + ops if cond_first else ops + cond_ops
    for k, (w, x) in enumerate(ops):
        nc.tensor.matmul(ps_m, w, x, start=(k == 0), stop=(k == total - 1),
                         perf_mode=DR)


def _pairs(ap4):
    """[128, 2, 2, N] AP -> per-pair [128, 2, N] moving APs."""
    return [ap4[:, 0, :, :], ap4[:, 1, :, :]]


@functools.lru_cache(maxsize=2)
def _program(biasfree=True, debug=False):
    nc = bacc.Bacc("TRN2", target_bir_lowering=False, debug=False,
                   num_devices=NCORES)

    app_d = nc.dram_tensor("app", [128, 4, 2, 8, 2, 256], FP8E4, kind="ExternalInput")
    mot_d = nc.dram_tensor("mot", [128, 8, 2, J], FP8E4, kind="ExternalInput")
    q_d = nc.dram_tensor("q", [128, 4, BS], BF16, kind="ExternalInput")
    wa_d = nc.dram_tensor("wa", [128, 8, 2, 512], FP8E4, kind="ExternalInput")
    wm_d = nc.dram_tensor("wm", [128, 8, 2, 512], FP8E4, kind="ExternalInput")
    wq_d = nc.dram_tensor("wq", [128, 4, 512], BF16, kind="ExternalInput")
    wvm_d = nc.dram_tensor("wvm", [128, 4, 512], FP8E3, kind="ExternalInput")
    wih_d = nc.dram_tensor("wih", [128, 16, 8, 2, 128], FP8E4, kind="ExternalInput")
    whh_d = nc.dram_tensor("whh", [128, 4, 2048], FP8E3, kind="ExternalInput")
    w1_d = nc.dram_tensor("w1", [128, 14, 2, 2, 2, 512], FP8E4, kind="ExternalInput")
    w2_d = nc.dram_tensor("w2", [128, 12, 2, 2, 2, 512], FP8E4, kind="ExternalInput")
    w2g_d = nc.dram_tensor("w2g", [128, 12, 2, 2, 2, 512], FP8E4, kind="ExternalInput")
    w3_d = nc.dram_tensor("w3", [128, 6, 2, 2, 2, 512], FP8E4, kind="ExternalInput")
    w4_d = nc.dram_tensor("w4", [128, 4, 8, 512], BF16, kind="ExternalInput")
    w4g_d = nc.dram_tensor("w4g", [128, 4, 8, 512], FP8E3, kind="ExternalInput")
    bias_d = nc.dram_tensor("bias", [128, NBIAS], F32, kind="ExternalInput")
    out_d = nc.dram_tensor("out", [128, 4, 4 * JV], BF16, kind="ExternalOutput")

    nc._phases = []

    def _mark(name):
        nc._phases.append((name, int(nc.get_next_instruction_name()[2:])))

    with tile.TileContext(nc) as tc:
        # Pools form a strict stack (release order = reverse of allocation).
        perm = tc.alloc_tile_pool(name="perm", bufs=1)
        tpool = tc.alloc_tile_pool(name="tmp", bufs=4)
        stream = tc.alloc_tile_pool(name="stream", bufs=4)
        p5 = tc.alloc_tile_pool(name="p5", bufs=1)        # clipT
        p4 = tc.alloc_tile_pool(name="p4", bufs=1)        # objs2T (+neg, S2)
        p3 = tc.alloc_tile_pool(name="p3", bufs=1)        # objsT (+neg, S1), condm
        p0 = tc.alloc_tile_pool(name="p0", bufs=1)        # early consts
        pp_early = tc.alloc_tile_pool(name="ps_early", bufs=1, space="PSUM")

        _mark("consts")
        bias = perm.tile([128, NBIAS], F32, name="bias")
        nc.sync.dma_start(bias, bias_d[:])

        def bap(off):
            return bias[:, off:off + 1]

        motT = p0.tile([128, 8, 2, J], FP8E4, name="motT")
        nc.sync.dma_start(motT, mot_d[:])
        qT = p0.tile([128, 4, BS], BF16, name="qT")
        nc.sync.dma_start(qT, q_d[:])
        wqt = p0.tile([128, 4, 512], BF16, name="wqt")
        nc.sync.dma_start(wqt, wq_d[:])

        _mark("qproj_condm")
        # ---------------- q_proj (bf16) -> qp [128, 4, BS]
        psq = pp_early.tile([128, 4, BS], F32, tag="psq", name="psq")
        for m in range(4):
            for kc in range(4):
                nc.tensor.matmul(psq[:, m, :], wqt[:, kc, m * 128:(m + 1) * 128],
                                 qT[:, kc, :], start=(kc == 0), stop=(kc == 3))
        qp = perm.tile([128, 4, BS], BF16, name="qp")
        if biasfree:
            nc.vector.tensor_copy(qp, psq)
        else:
            for m in range(4):
                nc.vector.tensor_scalar_add(qp[:, m, :], psq[:, m, :],
                                            bap(BOFF_Q + m))

        # ---------------- mot_proj (DR) -> condm [128, 2, 2, J] e4m3
        wmt = p0.tile([128, 8, 2, 512], FP8E4, name="wmt")
        nc.sync.dma_start(wmt, wm_d[:])
        pscm = pp_early.tile([128, 4, J], F32, tag="pscm", name="pscm")
        for m in range(4):
            for p in range(8):
                nc.tensor.matmul(pscm[:, m, :], wmt[:, p, :, m * 128:(m + 1) * 128],
                                 motT[:, p, :, :], start=(p == 0), stop=(p == 7),
                                 perf_mode=DR)
        condm = p3.tile([128, 2, 2, J], FP8E4, name="condm")
        condm_w = condm.rearrange("p a b j -> p (a b) j")
        if biasfree:
            nc.vector.tensor_scalar_mul(condm_w, pscm, IMM_CONDM)
        else:
            for m in range(4):
                nc.vector.tensor_scalar(condm_w[:, m, :], pscm[:, m, :],
                                        IMM_CONDM, bap(BOFF_M + m),
                                        OP.mult, OP.add)

        # cond broadcasts: condq e4m3 (x S_QP), qvc bf16
        condq = perm.tile([128, 2, 2, BS, C], FP8E4, name="condq")
        nc.vector.tensor_scalar_mul(
            condq.rearrange("p a b s c -> p (a b) s c"),
            qp[:, :, :, None].to_broadcast([128, 4, BS, C]), S_QP)
        qvc = perm.tile([128, 4, BS, T], BF16, name="qvc")
        nc.vector.tensor_copy(qvc, qp[:, :, :, None].to_broadcast([128, 4, BS, T]))
        qvc_v = qvc.rearrange("p d b t -> p d (b t)")
        pp_early.release()

        _mark("stageA")
        # ---------------- stage A: app_proj (DR) -> objsT/neg [128,2,2,F,J]
        p2 = tc.alloc_tile_pool(name="p2", bufs=1)
        apps = tc.alloc_tile_pool(name="apps", bufs=2)
        pp_a = tc.alloc_tile_pool(name="ps_a", bufs=2, space="PSUM")
        wat = p2.tile([128, 8, 2, 512], FP8E4, name="wat")
        nc.sync.dma_start(wat, wa_d[:])
        objsT = p3.tile([128, 2, 2, F, J], FP8E4, name="objsT")
        nobjsT = p3.tile([128, 2, 2, F, J], FP8E4, name="nobjsT")
        for cc in range(4):
            xc = apps.tile([128, 2, 8, 2, 256], FP8E4, tag="app", name="xc")
            nc.sync.dma_start(xc, app_d[:, cc, :, :, :, :])
            for m in range(4):
                ps_a = pp_a.tile([128, 512], F32, tag="psA", name="ps_a")
                for h in range(2):
                    for p in range(8):
                        nc.tensor.matmul(ps_a[:, h * 256:(h + 1) * 256],
                                         wat[:, p, :, m * 128:(m + 1) * 128],
                                         xc[:, h, p, :, :],
                                         start=(p == 0), stop=(p == 7),
                                         perf_mode=DR)
                dst = objsT[:, m // 2, m % 2, cc * 4:(cc + 1) * 4, :]
                dst = dst.rearrange("p f j -> p (f j)")
                ndst = nobjsT[:, m // 2, m % 2, cc * 4:(cc + 1) * 4, :]
                ndst = ndst.rearrange("p f j -> p (f j)")
                if biasfree:
                    nc.vector.tensor_scalar_mul(dst, ps_a, IMM_OBJS)
                else:
                    nc.vector.tensor_scalar(dst, ps_a, IMM_OBJS,
                                            bap(BOFF_A + m), OP.mult, OP.add)
                nc.scalar.mul(ndst, dst, -1.0)
        pp_a.release()
        apps.release()
        p2.release()

        # S1 = sum_f objsT (two-accumulator bf16 chain, final e4m3)
        s1 = p3.tile([128, 2, 2, J], FP8E4, name="s1")
        s1a = p3.tile([128, 2, 2, J], BF16, name="s1a")
        s1b = p3.tile([128, 2, 2, J], BF16, name="s1b")
        nc.vector.tensor_add(s1a, objsT[:, :, :, 0, :], objsT[:, :, :, 1, :])
        nc.vector.tensor_add(s1b, objsT[:, :, :, 2, :], objsT[:, :, :, 3, :])
        for f in range(4, F):
            t = s1a if f % 2 == 0 else s1b
            nc.vector.tensor_add(t, t, objsT[:, :, :, f, :])
        nc.vector.tensor_add(s1, s1a, s1b)

        _mark("crn_m")
        # ---------------- crn_m: objsT -> objs2T [128, 2, 2, 14, J]
        pp_crn = tc.alloc_tile_pool(name="ps_crn", bufs=2, space="PSUM")
        objs2T = p4.tile([128, 2, 2, 14, J], FP8E4, name="objs2T")
        nobjs2T = p4.tile([128, 2, 2, 14, J], FP8E4, name="nobjs2T")
        s2a = p4.tile([128, 2, 2, J], BF16, name="s2a")

        def obj1(f):
            return _pairs(objsT[:, :, :, f, :])

        def nobj1(f):
            return _pairs(nobjsT[:, :, :, f, :])

        # complement scales last so S1/neg have time to complete
        order_m = ([i for i, s in enumerate(SELS_M) if not _use_comp(F, s)]
                   + [i for i, s in enumerate(SELS_M) if _use_comp(F, s)])
        cond_m_pairs = _pairs(condm)
        for oi, si in enumerate(order_m):
            sel = SELS_M[si]
            w1t = stream.tile([128, 2, 2, 2, 512], FP8E4, tag="crnw", name="w1t", bufs=8)
            nc.sync.dma_start(w1t, w1_d[:, si, :, :, :, :])
            if _use_comp(F, sel):
                in_set = set(sel)
                units = [_pairs(s1)] + [nobj1(i) for i in range(F)
                                        if i not in in_set]
            else:
                units = [obj1(f) for f in sel]
            ps = pp_crn.tile([128, 4, J], F32, tag="psM", name="ps_m1", bufs=3)
            for m in range(4):
                _dr_group(nc, ps[:, m, :], w1t[:, 0], w1t[:, 1], units,
                          cond_m_pairs, slice(m * 128, (m + 1) * 128))
            # epilogue: objs2T[si] = S_OBJ2 * elu(inv1 * ps + b)
            t_e = tpool.tile([128, 4, J], BF16, tag="t_exp", name="t_e", bufs=2)
            t_m = tpool.tile([128, 4, J], BF16, tag="t_min", name="t_m", bufs=2)
            dst = objs2T[:, :, :, si, :].rearrange("p a b j -> p (a b) j")
            ndst = nobjs2T[:, :, :, si, :].rearrange("p a b j -> p (a b) j")
            if biasfree:
                nc.scalar.activation(t_e, ps, AF.Exp, bias=bap(COL_LN2),
                                     scale=INV_1)
                nc.vector.tensor_scalar(t_m, t_e, S_OBJ2, -S_OBJ2, OP.min, OP.add)
                nc.vector.scalar_tensor_tensor(dst, ps, IMM_1, t_m,
                                               OP.mult, OP.max)
            else:
                for m in range(4):
                    nc.scalar.activation(t_e[:, m, :], ps[:, m, :], AF.Exp,
                                         bias=bap(BOFF_1 + si * 4 + m),
                                         scale=INV_1)
                nc.vector.tensor_scalar(t_m, t_e, S_OBJ2, -S_OBJ2, OP.min, OP.add)
                for m in range(4):
                    lin = tpool.tile([128, J], F32, tag="lin", name="lin")
                    nc.vector.tensor_scalar(lin, ps[:, m, :], IMM_1,
                                            bap(BOFF_1L + si * 4 + m),
                                            OP.mult, OP.add)
                    nc.vector.tensor_tensor(dst[:, m, :], lin, t_m[:, m, :], OP.max)
            nc.scalar.mul(ndst, dst, -1.0)
            # incremental S2
            s2src = objs2T[:, :, :, si, :]
            if oi == 0:
                nc.vector.tensor_copy(s2a, s2src)
            else:
                nc.vector.tensor_add(s2a, s2a, s2src)
        s2 = p4.tile([128, 2, 2, J], FP8E4, name="s2")
        nc.vector.tensor_copy(s2, s2a)

        _mark("gatesx")
        # ---------------- LSTM x-gates: gx = inv * (W_ih @ motT)  (DR)
        wihs = tc.alloc_tile_pool(name="wihs", bufs=10)
        p1 = tc.alloc_tile_pool(name="p1", bufs=1)
        ppx = tc.alloc_tile_pool(name="ps_x", bufs=2, space="PSUM")
        whht = p1.tile([128, 4, 2048], FP8E3, name="whht")
        nc.sync.dma_start(whht, whh_d[:])
        wvmt = p1.tile([128, 4, 512], FP8E3, name="wvmt")
        nc.sync.dma_start(wvmt, wvm_d[:])
        gx = p1.tile([128, 16, J], F32, name="gx")
        for mi in range(16):
            wih_t = wihs.tile([128, 8, 2, 128], FP8E4, tag="wih", name="wih_t")
            nc.sync.dma_start(wih_t, wih_d[:, mi, :, :, :])
            psx = ppx.tile([128, J], F32, tag="psx", name="psx")
            for p in range(8):
                nc.tensor.matmul(psx, wih_t[:, p, :, :], motT[:, p, :, :],
                                 start=(p == 0), stop=(p == 7), perf_mode=DR)
            if biasfree:
                nc.vector.tensor_scalar_mul(gx[:, mi, :], psx, INV_GX)
            else:
                nc.vector.tensor_scalar(gx[:, mi, :], psx, INV_GX,
                                        bap(BOFF_G + mi), OP.mult, OP.add)
        ppx.release()
        pp_r = tc.alloc_tile_pool(name="ps_r", bufs=2, space="PSUM")
        gxr = gx.rearrange("p m (b c) -> p m c b", c=C)

        _mark("lstm")
        # ---------------- LSTM recurrence, tanh-only form (one act table):
        # sigmoid(x) = (tanh(x/2)+1)/2. Track c2 = 2c and h2 = 2h; the /2 of
        # each sigmoid folds into stt imms and W_hh/Wvm are pre-halved on the
        # host so psr = W_hh @ h exactly.
        h_prev = None
        c_prev = None
        for t in range(C):
            xg = gxr[:, :, t, :]
            if t == 0:
                gates = xg
            else:
                psr = pp_r.tile([128, 16, BS], F32, tag="psr", name="psr")
                for mi in range(16):
                    for kc in range(4):
                        nc.tensor.matmul(psr[:, mi, :],
                                         whht[:, kc, mi * 128:(mi + 1) * 128],
                                         h_prev[:, kc, :],
                                         start=(kc == 0), stop=(kc == 3))
                gates = tpool.tile([128, 16, BS], F32, tag="lstm_g", name="lstm_g")
                nc.vector.scalar_tensor_tensor(gates, psr, INV_HH, xg,
                                               OP.mult, OP.add)
            t_if = tpool.tile([128, 8, BS], F32, tag="dif", name="t_if")
            nc.scalar.activation(t_if, gates[:, 0:8, :], AF.Tanh, scale=0.5)
            tan_g = tpool.tile([128, 4, BS], F32, tag="tg", name="tan_g")
            nc.scalar.activation(tan_g, gates[:, 8:12, :], AF.Tanh)
            t_o = tpool.tile([128, 4, BS], F32, tag="do", name="t_o")
            nc.scalar.activation(t_o, gates[:, 12:16, :], AF.Tanh, scale=0.5)
            # ig2 = (tanh_i+1)*tan_g = 2*sig_i*tan_g
            ig2 = tpool.tile([128, 4, BS], F32, tag="ig", name="ig2", bufs=2)
            nc.vector.scalar_tensor_tensor(ig2, t_if[:, 0:4, :], 1.0, tan_g,
                                           OP.add, OP.mult)
            if t == 0:
                c2_t = ig2
            else:
                # fc2 = (tanh_f+1)*c2_prev = 4*sig_f*c ; c2 = fc2/2 + ig2
                c2_t = tpool.tile([128, 4, BS], F32, tag="c_t", name="c2_t", bufs=2)
                fc2 = tpool.tile([128, 4, BS], F32, tag="fc", name="fc2")
                nc.vector.scalar_tensor_tensor(fc2, t_if[:, 4:8, :], 1.0,
                                               c_prev, OP.add, OP.mult)
                nc.vector.scalar_tensor_tensor(c2_t, fc2, 0.5, ig2,
                                               OP.mult, OP.add)
            tan_c = tpool.tile([128, 4, BS], F32, tag="tanc", name="tan_c")
            nc.scalar.activation(tan_c, c2_t, AF.Tanh, scale=0.5)
            # h2 = (tanh_o+1)*tanh(c) = 2h
            h_t = tpool.tile([128, 4, BS], BF16, tag="h_t", name="h2_t", bufs=2)
            nc.vector.scalar_tensor_tensor(h_t, t_o, 1.0, tan_c,
                                           OP.add, OP.mult)
            h_prev, c_prev = h_t, c2_t

        # vm_proj (bf16) -> vmc [128, 2, 2, BS, T] e4m3 (x S_VMP)
        psv = pp_r.tile([128, 4, BS], F32, tag="psv", name="psv", bufs=1)
        for m in range(4):
            for kc in range(4):
                nc.tensor.matmul(psv[:, m, :], wvmt[:, kc, m * 128:(m + 1) * 128],
                                 h_prev[:, kc, :], start=(kc == 0), stop=(kc == 3))
        vmp = p1.tile([128, 4, BS], FP8E4, name="vmp")
        if biasfree:
            nc.vector.tensor_scalar_mul(vmp, psv, S_VMP * INV_HH)
        else:
            for m in range(4):
                nc.vector.tensor_scalar(vmp[:, m, :], psv[:, m, :], S_VMP * INV_HH,
                                        bap(BOFF_VM + m), OP.mult, OP.add)
        vmc = perm.tile([128, 2, 2, BS, T], FP8E4, name="vmc")
        nc.vector.tensor_copy(
            vmc.rearrange("p a b s t -> p (a b) s t"),
            vmp[:, :, :, None].to_broadcast([128, 4, BS, T]))
        pp_r.release()
        p1.release()
        wihs.release()

        _mark("crn_q")
        # ---------------- crn_q (gated): objs2T -> clipT [128,2,2,C,BS,T]
        clipT = p5.tile([128, 2, 2, C, BS, T], FP8E4, name="clipT")

        def obj2(s):
            return _pairs(objs2T[:, :, :, s, :])

        def nobj2(s):
            return _pairs(nobjs2T[:, :, :, s, :])

        order_q = ([i for i, s in enumerate(SELS_Q) if not _use_comp(F - 2, s)]
                   + [i for i, s in enumerate(SELS_Q) if _use_comp(F - 2, s)])
        condq_pairs = _pairs(condq.rearrange("p a b s c -> p a b (s c)"))
        for si in order_q:
            sel = SELS_Q[si]
            w2t = stream.tile([128, 2, 2, 2, 512], FP8E4, tag="crnw", name="w2t", bufs=8)
            nc.sync.dma_start(w2t, w2_d[:, si, :, :, :, :])
            w2gt = stream.tile([128, 2, 2, 2, 512], FP8E4, tag="crnw", name="w2gt", bufs=8)
            nc.sync.dma_start(w2gt, w2g_d[:, si, :, :, :, :])
            if _use_comp(F - 2, sel):
                in_set = set(sel)
                units = [_pairs(s2)] + [nobj2(i) for i in range(F - 2)
                                        if i not in in_set]
            else:
                units = [obj2(s) for s in sel]
            ps_m = pp_crn.tile([128, 4, J], F32, tag="psM", name="ps_q1", bufs=3)
            ps_g = pp_crn.tile([128, 4, J], F32, tag="psG", name="ps_q2")
            for m in range(4):
                _dr_group(nc, ps_m[:, m, :], w2t[:, 0], w2t[:, 1], units,
                          condq_pairs, slice(m * 128, (m + 1) * 128))
            for m in range(4):
                _dr_group(nc, ps_g[:, m, :], w2gt[:, 0], w2gt[:, 1], units,
                          condq_pairs, slice(m * 128, (m + 1) * 128))
            # gated epilogue, tanh form: t_z carries S_OBJ3/2 * elu;
            # out = (tanh(gate/2)+1) * t_z = S_OBJ3 * elu * sigmoid(gate)
            t_e = tpool.tile([128, 4, J], BF16, tag="t_exp", name="t_e", bufs=2)
            t_m = tpool.tile([128, 4, J], BF16, tag="t_min", name="t_m", bufs=2)
            t_z = tpool.tile([128, 4, J], BF16, tag="t_z", name="t_z", bufs=2)
            t_d = tpool.tile([128, 4, J], BF16, tag="t_d", name="t_d", bufs=2)
            h3 = S_OBJ3 / 2
            if biasfree:
                nc.scalar.activation(t_e, ps_m, AF.Exp, bias=bap(COL_LN3),
                                     scale=INV_2)
                nc.vector.tensor_scalar(t_m, t_e, h3, -h3, OP.min, OP.add)
                nc.vector.scalar_tensor_tensor(t_z, ps_m, IMM_2 / 2, t_m,
                                               OP.mult, OP.max)
                nc.scalar.activation(t_d, ps_g, AF.Tanh, scale=INV_2 / 2)
            else:
                for m in range(4):
                    nc.scalar.activation(t_e[:, m, :], ps_m[:, m, :], AF.Exp,
                                         bias=bap(BOFF_2 + si * 4 + m),
                                         scale=INV_2)
                    nc.scalar.activation(t_d[:, m, :], ps_g[:, m, :], AF.Tanh,
                                         bias=bap(BOFF_G2 + si * 4 + m),
                                         scale=INV_2 / 2)
                nc.vector.tensor_scalar(t_m, t_e, h3, -h3, OP.min, OP.add)
                for m in range(4):
                    lin = tpool.tile([128, J], F32, tag="lin", name="lin")
                    nc.vector.tensor_scalar(lin, ps_m[:, m, :], IMM_2 / 2,
                                            bap(BOFF_2L + si * 4 + m),
                                            OP.mult, OP.add)
                    nc.vector.tensor_tensor(t_z[:, m, :], lin, t_m[:, m, :], OP.max)
            # dst view: cols j=(b c) -> clipT[:, :, :, c, b, si].
            # (t_d + 1) via a 3D-out tensor_scalar first: ScalarTensorTensor
            # outputs must be <= 3D and the clipT view is 4D.
            nc.vector.tensor_scalar_add(t_d, t_d, 1.0)
            wide = clipT[:, :, :, :, :, si].rearrange("p a b c s -> p (a b) s c")
            nc.vector.tensor_tensor(wide, t_d.rearrange("p d (s c) -> p d s c", c=C),
                                    t_z.rearrange("p d (s c) -> p d s c", c=C),
                                    OP.mult)
        pp_crn.release()
        p0.release()
        p3.release()
        p4.release()

        _mark("crn_vm")
        # ---------------- crn_vm (ungated, direct sums): clipT -> objs4T bf16
        pp_v = tc.alloc_tile_pool(name="ps_v", bufs=1, space="PSUM")
        objs4T = perm.tile([128, 4, 6, JV], BF16, name="objs4T")

        def clip_pairs(c):
            ap = clipT[:, :, :, c, :, :]
            return _pairs(ap.rearrange("p a b s t -> p a b (s t)"))

        vmc_pairs = _pairs(vmc.rearrange("p a b s t -> p a b (s t)"))
        # incremental S4 accumulator (bf16) so crn_vq's complement scales can
        # start right after the last crn_vm epilogue
        s4 = perm.tile([128, 4, JV], BF16, name="s4")
        for si, sel in enumerate(SELS_VM):
            w3t = stream.tile([128, 2, 2, 2, 512], FP8E4, tag="crnw", name="w3t", bufs=8)
            nc.sync.dma_start(w3t, w3_d[:, si, :, :, :, :])
            units = [clip_pairs(c) for c in sel]
            ps0 = pp_v.tile([128, 2, JV], F32, tag="psV0", name="ps_vm0", bufs=2)
            ps1 = pp_v.tile([128, 2, JV], F32, tag="psV1", name="ps_vm1", bufs=2)
            ps_list = [ps0[:, 0, :], ps0[:, 1, :], ps1[:, 0, :], ps1[:, 1, :]]
            for m in range(4):
                _dr_group(nc, ps_list[m], w3t[:, 0], w3t[:, 1], units,
                          vmc_pairs, slice(m * 128, (m + 1) * 128))
            for half, psh in enumerate((ps0, ps1)):
                t_e = tpool.tile([128, 2, JV], BF16, tag="t_expv", name="t_ev", bufs=2)
                t_m = tpool.tile([128, 2, JV], BF16, tag="t_minv", name="t_mv", bufs=2)
                dst = objs4T[:, half * 2:(half + 1) * 2, si, :]
                if biasfree:
                    nc.scalar.activation(t_e, psh, AF.Exp, scale=INV_3)
                    nc.vector.tensor_scalar(t_m, t_e, 1.0, -1.0, OP.min, OP.add)
                    nc.vector.scalar_tensor_tensor(dst, psh, INV_3, t_m,
                                                   OP.mult, OP.max)
                else:
                    for mm in range(2):
                        m = half * 2 + mm
                        nc.scalar.activation(t_e[:, mm, :], psh[:, mm, :], AF.Exp,
                                             bias=bap(BOFF_3 + si * 4 + m),
                                             scale=INV_3)
                        nc.vector.tensor_scalar(t_m[:, mm, :], t_e[:, mm, :],
                                                1.0, -1.0, OP.min, OP.add)
                        lin = tpool.tile([128, JV], F32, tag="linv", name="linv")
                        nc.vector.tensor_scalar(lin, psh[:, mm, :], INV_3,
                                                bap(BOFF_3L + si * 4 + m),
                                                OP.mult, OP.add)
                        nc.vector.tensor_tensor(dst[:, mm, :], lin, t_m[:, mm, :],
                                                OP.max)
            # incremental S4
            s4src = objs4T[:, :, si, :]
            if si == 0:
                nc.vector.tensor_copy(s4, s4src)
            else:
                nc.vector.tensor_add(s4, s4, s4src)

        _mark("crn_vq")
        # ---------------- crn_vq (bf16, gated): objs4T -> out
        def o4_slice(s):
            return objs4T[:, :, s, :]

        gpool = tc.alloc_tile_pool(name="gpool", bufs=4)
        # direct-sum scales first: they don't need s4
        order_vq = ([i for i, s in enumerate(SELS_VQ)
                     if not (C - 2 - len(s)) + 1 < len(s)]
                    + [i for i, s in enumerate(SELS_VQ)
                       if (C - 2 - len(s)) + 1 < len(s)])
        for si in order_vq:
            sel = SELS_VQ[si]
            w4t = stream.tile([128, 8, 512], BF16, tag="crnw4", name="w4t", bufs=2)
            nc.sync.dma_start(w4t, w4_d[:, si, :, :])
            w4gt = stream.tile([128, 8, 512], FP8E3, tag="crnw4g", name="w4gt", bufs=2)
            nc.sync.dma_start(w4gt, w4g_d[:, si, :, :])
            # g = subset sum (bf16 DVE, complement vs direct)
            in_set = set(sel)
            comp = [i for i in range(C - 2) if i not in in_set]
            if len(comp) + 1 < len(sel):
                g = gpool.tile([128, 4, JV], BF16, tag="g4", name="g4")
                nc.vector.tensor_sub(g, s4, o4_slice(comp[0]))
                for i in comp[1:]:
                    nc.vector.tensor_sub(g, g, o4_slice(i))
            elif len(sel) == 1:
                g = o4_slice(sel[0])
            else:
                g = gpool.tile([128, 4, JV], BF16, tag="g4", name="g4")
                nc.vector.tensor_add(g, o4_slice(sel[0]), o4_slice(sel[1]))
                for i in sel[2:]:
                    nc.vector.tensor_add(g, g, o4_slice(i))
            ps0 = pp_v.tile([128, 2, JV], F32, tag="psV0", name="ps_vq0", bufs=2)
            ps1 = pp_v.tile([128, 2, JV], F32, tag="psV1", name="ps_vq1", bufs=2)
            pg0 = pp_v.tile([128, 2, JV], F32, tag="psV2", name="ps_vq2", bufs=2)
            pg1 = pp_v.tile([128, 2, JV], F32, tag="psV3", name="ps_vq3", bufs=2)
            ps_list = [ps0[:, 0, :], ps0[:, 1, :], ps1[:, 0, :], ps1[:, 1, :]]
            pg_list = [pg0[:, 0, :], pg0[:, 1, :], pg1[:, 0, :], pg1[:, 1, :]]
            for m in range(4):
                msl = slice(m * 128, (m + 1) * 128)
                for kc in range(4):
                    nc.tensor.matmul(ps_list[m], w4t[:, 4 + kc, msl],
                                     qvc_v[:, kc, :], start=(kc == 0), stop=False)
                for kc in range(4):
                    nc.tensor.matmul(ps_list[m], w4t[:, kc, msl], g[:, kc, :],
                                     start=False, stop=(kc == 3))
            for m in range(4):
                msl = slice(m * 128, (m + 1) * 128)
                for kc in range(4):
                    nc.tensor.matmul(pg_list[m], w4gt[:, 4 + kc, msl],
                                     qvc_v[:, kc, :], start=(kc == 0), stop=False)
                for kc in range(4):
                    nc.tensor.matmul(pg_list[m], w4gt[:, kc, msl], g[:, kc, :],
                                     start=False, stop=(kc == 3))
            ot = tpool.tile([128, 4, JV], BF16, tag="ot", name="ot4", bufs=2)
            for half, (psh, pgh) in enumerate(((ps0, pg0), (ps1, pg1))):
                t_e = tpool.tile([128, 2, JV], BF16, tag="t_expv", name="t_ev", bufs=2)
                t_m = tpool.tile([128, 2, JV], BF16, tag="t_minv", name="t_mv", bufs=2)
                t_z = tpool.tile([128, 2, JV], BF16, tag="t_zv", name="t_zv", bufs=2)
                t_d = tpool.tile([128, 2, JV], BF16, tag="t_dv", name="t_dv", bufs=2)
                oth = ot[:, half * 2:(half + 1) * 2, :]
                if biasfree:
                    nc.scalar.activation(t_e, psh, AF.Exp, bias=bap(COL_LNH))
                    nc.vector.tensor_scalar(t_m, t_e, 0.5, -0.5, OP.min, OP.add)
                    nc.vector.scalar_tensor_tensor(t_z, psh, 0.5, t_m,
                                                   OP.mult, OP.max)
                    nc.scalar.activation(t_d, pgh, AF.Tanh, scale=INV_G4 / 2)
                    nc.vector.scalar_tensor_tensor(oth, t_d, 1.0, t_z,
                                                   OP.add, OP.mult)
                else:
                    for mm in range(2):
                        m = half * 2 + mm
                        nc.scalar.activation(t_e[:, mm, :], psh[:, mm, :], AF.Exp,
                                             bias=bap(BOFF_4 + si * 4 + m))
                        nc.scalar.activation(t_d[:, mm, :], pgh[:, mm, :],
                                             AF.Tanh,
                                             bias=bap(BOFF_G4 + si * 4 + m),
                                             scale=INV_G4 / 2)
                        nc.vector.tensor_scalar(t_m[:, mm, :], t_e[:, mm, :],
                                                0.5, -0.5, OP.min, OP.add)
                        lin = tpool.tile([128, JV], F32, tag="linv", name="linv")
                        nc.vector.tensor_scalar(lin, psh[:, mm, :], 0.5,
                                                bap(BOFF_4L + si * 4 + m),
                                                OP.mult, OP.add)
                        nc.vector.tensor_tensor(t_z[:, mm, :], lin, t_m[:, mm, :],
                                                OP.max)
                        nc.vector.scalar_tensor_tensor(oth[:, mm, :],
                                                       t_d[:, mm, :], 1.0,
                                                       t_z[:, mm, :],
                                                       OP.add, OP.mult)
            nc.sync.dma_start(out_d[:, si, :], ot.rearrange("p d j -> p (d j)"))

        for pool in (gpool, pp_v, p5, stream, tpool, perm):
            pool.release()

    nc.compile()
    return nc


# ---------------------------------------------------------------- host side


def _kxm_pairs(w_t, scale, dt):
    """[K, M] f32 -> [128, K//256, 2, M] scaled/clipped fp8 (pair-grouped)."""
    K, M = w_t.shape
    lim = 240.0 if dt is E4 else 15.5
    w = np.clip(np.asarray(w_t, np.float32) * scale, -lim, lim)
    return np.ascontiguousarray(
        w.reshape(K // 256, 2, 128, M).transpose(2, 0, 1, 3)).astype(dt)


def _bank_dr(Ws, sels, s_cond_ratio):
    """CRN bank -> [128, S, 2, 2, 2, 512] e4m3: halves (g/|sel|, c*ratio)."""
    per = []
    for si, sel in enumerate(sels):
        s_id = si + 1
        hg = np.asarray(Ws[s_id][:, :D], np.float32).T / len(sel) * SW
        hc = np.asarray(Ws[s_id][:, D:], np.float32).T * (SW * s_cond_ratio)
        h = np.stack([hg, hc])  # [2, 512, 512]
        h = np.clip(h, -240, 240)
        per.append(h.reshape(2, 2, 2, 128, 512).transpose(3, 0, 1, 2, 4))
    return np.ascontiguousarray(np.stack(per, axis=1)).astype(E4)


def _bank_bf16(Ws, sels, dt=BF, scale=1.0):
    """Stage-4 bank -> [128, S, 8, 512] (halves g/|sel|, c as 4+4 k-chunks)."""
    lim = {BF: 3e38, E3: 15.5, E4: 240.0}[dt]
    per = []
    for si, sel in enumerate(sels):
        s_id = si + 1
        hg = np.asarray(Ws[s_id][:, :D], np.float32).T / len(sel) * scale
        hc = np.asarray(Ws[s_id][:, D:], np.float32).T * scale
        h = np.concatenate([hg, hc], axis=0)  # [1024, 512]
        h = np.clip(h, -lim, lim)
        per.append(h.reshape(8, 128, 512).transpose(1, 0, 2))
    return np.ascontiguousarray(np.stack(per, axis=1)).astype(dt)


def _kxm_e3(w_t, kchunks):
    K, M = w_t.shape
    w = np.clip(np.asarray(w_t, np.float32) * SW3, -15.5, 15.5)
    return np.ascontiguousarray(
        w.reshape(kchunks, 128, M).transpose(1, 0, 2)).astype(E3)


def _to_kxm_bf16(w_t, kchunks):
    K, M = w_t.shape
    return np.ascontiguousarray(
        np.asarray(w_t, np.float32).reshape(kchunks, 128, M)
        .transpose(1, 0, 2)).astype(BF)


def _vec_to_pm(v, chunks):
    return np.ascontiguousarray(np.asarray(v, np.float32).reshape(chunks, 128).T)


def _prep_weights(inputs):
    w = {}
    w["wa"] = _kxm_pairs(np.asarray(inputs["Wa"], np.float32).T, SW, E4)
    w["wm"] = _kxm_pairs(np.asarray(inputs["Wm"], np.float32).T, SW, E4)
    w["wq"] = _to_kxm_bf16(np.asarray(inputs["Wq"], np.float32).T, 4)
    # Wvm/W_hh halved (device tracks h2 = 2h), shipped e3m4 x 64
    w["wvm"] = _kxm_e3(np.asarray(inputs["Wvm"], np.float32).T / 2, 4)
    wih = _kxm_pairs(np.asarray(inputs["W_ih"], np.float32).T, SW, E4)
    # [128, 8, 2, 2048] -> [128, mi 16, pair 8, 2, 128]
    w["wih"] = np.ascontiguousarray(
        wih.reshape(128, 8, 2, 16, 128).transpose(0, 3, 1, 2, 4))
    w["whh"] = _kxm_e3(np.asarray(inputs["W_hh"], np.float32).T / 2, 4)
    w["w1"] = _bank_dr(np.asarray(inputs["W1"], np.float32), SELS_M,
                       S_OBJ1 / S_CONDM)
    w["w2"] = _bank_dr(np.asarray(inputs["W2"], np.float32), SELS_Q,
                       S_OBJ2 / S_QP)
    w["w2g"] = _bank_dr(np.asarray(inputs["gW2"], np.float32), SELS_Q,
                        S_OBJ2 / S_QP)
    w["w3"] = _bank_dr(np.asarray(inputs["W3"], np.float32), SELS_VM,
                       S_OBJ3 / S_VMP)
    w["w4"] = _bank_bf16(np.asarray(inputs["W4"], np.float32), SELS_VQ)
    w["w4g"] = _bank_bf16(np.asarray(inputs["gW4"], np.float32), SELS_VQ,
                          dt=E3, scale=SW3)

    bias = np.zeros((128, NBIAS), np.float32)
    bias[:, COL_LN2] = np.log(S_OBJ2)
    bias[:, COL_LN3] = np.log(S_OBJ3 / 2)
    bias[:, COL_LNH] = np.log(0.5)
    bias[:, BOFF_A:BOFF_A + 4] = _vec_to_pm(inputs["ba"], 4) * S_OBJ1
    bias[:, BOFF_M:BOFF_M + 4] = _vec_to_pm(inputs["bm"], 4) * S_CONDM
    bias[:, BOFF_Q:BOFF_Q + 4] = _vec_to_pm(inputs["bq"], 4)
    bias[:, BOFF_VM:BOFF_VM + 4] = _vec_to_pm(inputs["bvm"], 4) * S_VMP
    bias[:, BOFF_G:BOFF_G + 16] = _vec_to_pm(
        np.asarray(inputs["b_ih"], np.float32)
        + np.asarray(inputs["b_hh"], np.float32), 16)
    for si in range(len(SELS_M)):
        b = _vec_to_pm(inputs["b1"][si + 1], 4)
        bias[:, BOFF_1 + si * 4:BOFF_1 + si * 4 + 4] = b + np.log(S_OBJ2)
        bias[:, BOFF_1L + si * 4:BOFF_1L + si * 4 + 4] = b * S_OBJ2
    for si in range(len(SELS_Q)):
        b = _vec_to_pm(inputs["b2"][si + 1], 4)
        bias[:, BOFF_2 + si * 4:BOFF_2 + si * 4 + 4] = b + np.log(S_OBJ3 / 2)
        bias[:, BOFF_2L + si * 4:BOFF_2L + si * 4 + 4] = b * (S_OBJ3 / 2)
        bias[:, BOFF_G2 + si * 4:BOFF_G2 + si * 4 + 4] = _vec_to_pm(
            inputs["gb2"][si + 1], 4) / 2
    for si in range(len(SELS_VM)):
        b = _vec_to_pm(inputs["b3"][si + 1], 4)
        bias[:, BOFF_3 + si * 4:BOFF_3 + si * 4 + 4] = b
        bias[:, BOFF_3L + si * 4:BOFF_3L + si * 4 + 4] = b
    for si in range(len(SELS_VQ)):
        b = _vec_to_pm(inputs["b4"][si + 1], 4)
        bias[:, BOFF_4 + si * 4:BOFF_4 + si * 4 + 4] = b + np.log(0.5)
        bias[:, BOFF_4L + si * 4:BOFF_4L + si * 4 + 4] = b / 2
        bias[:, BOFF_G4 + si * 4:BOFF_G4 + si * 4 + 4] = _vec_to_pm(
            inputs["gb4"][si + 1], 4) / 2
    w["bias"] = bias
    return w


def _prep_core_inputs(inputs, core):
    b0 = core * BS
    app = np.asarray(inputs["appearance_video_feat"][b0:b0 + BS], np.float32)
    mot = np.asarray(inputs["motion_video_feat"][b0:b0 + BS], np.float32)
    q = np.asarray(inputs["question_embedding"][b0:b0 + BS], np.float32)
    # app [BS, C, F, V] -> [p, cc, half, pair, i, (f_h j)] e4m3 (x S_APP)
    app_t = app.transpose(3, 2, 0, 1).reshape(V, F, J)          # [V, F, J]
    app_t = app_t.reshape(8, 2, 128, F, J).transpose(2, 0, 1, 3, 4)  # [p,pr,i,F,J]
    app_t = app_t.reshape(128, 8, 2, 4, 2, 2, J)                # F -> cc,h,f_h
    app_t = app_t.transpose(0, 3, 4, 1, 2, 5, 6).reshape(128, 4, 2, 8, 2, 256)
    app_t = np.clip(app_t * S_APP, -240, 240)
    # mot [BS, C, V] -> [p, pair, i, j] e4m3 (x S_MOT)
    mot_t = mot.transpose(2, 0, 1).reshape(V, J)
    mot_t = mot_t.reshape(8, 2, 128, J).transpose(2, 0, 1, 3)
    mot_t = np.clip(mot_t * S_MOT, -240, 240)
    q_t = q.T.reshape(4, 128, BS).transpose(1, 0, 2)
    return {
        "app": np.ascontiguousarray(app_t).astype(E4),
        "mot": np.ascontiguousarray(mot_t).astype(E4),
        "q": np.ascontiguousarray(q_t).astype(BF),
    }


def _assemble(results):
    out = np.empty((B, (C - 4) * T, D), np.float32)
    for core in range(NCORES):
        r = np.asarray(results[core]["out"], np.float32).reshape(128, 4, 4, BS, T)
        # [p, si, m, b, t] -> [b, si, t, m, p]
        o = r.transpose(3, 1, 4, 2, 0).reshape(BS, (C - 4) * T, D)
        out[core * BS:(core + 1) * BS] = o
    return out


def build_in_maps(**inputs):
    w = _prep_weights(inputs)
    in_maps = []
    for core in range(NCORES):
        m = dict(w)
        m.update(_prep_core_inputs(inputs, core))
        in_maps.append(m)
    return in_maps


def _all_biases_zero(inputs):
    names = ["ba", "bm", "bq", "bvm", "b_ih", "b_hh", "b1", "b2", "gb2",
             "b3", "b4", "gb4"]
    return all(not np.any(np.asarray(inputs[n], np.float32)) for n in names)


def kernel(**inputs):
    nc = _program(_all_biases_zero(inputs))
    in_maps = build_in_maps(**inputs)
    res = run_bass_kernel_spmd(nc, in_maps, list(range(NCORES)))
    return _assemble(res.results)


if __name__ == "__main__":
    import reference

    inputs = {k: np.asarray(v) for k, v in reference.setup_inputs().items()}
    out = kernel(**inputs)
    exp = np.asarray(reference.reference(**inputs))
    err = np.abs(out - exp).max() / np.abs(exp).max()
    print("Relative error:", err)


# revision 10
# speedup vs baseline: 1.0134x; 1.0065x over previous
"""Trainium2 Bass kernel for nn_EncoderVidCRN (CRN video QA encoder), fp8 rev.

Data parallel over batch B=128 across 8 NeuronCores (16 rows/core). Mixed
precision chosen from a measured per-tensor error budget (rel gate 2e-2):

 - fp8-e4m3 (DoubleRow matmuls, 2 K-chunks/instruction): appearance proj
   (app, Wa), motion proj (mot, Wm), LSTM x-gates (W_ih), clip CRN banks
   W1/W2/gW2 and their moving operands (objsT, condm, condq, objs2T), video
   CRN-1 bank W3 with moving (clipT, vmc). Weight scale 2048, activation
   scales are fixed powers of two; inverse scales fold into epilogue imms.
 - fp8-e3m4 (plain matmuls, stationary only, scale 64): gW4, W_hh, Wvm.
 - bf16: q path (Wq) and the entire final CRN stage (W4, objs4T, qvc) --
   the only error-sensitive paths (measured: Wq or W4/mov4 in fp8 alone
   would each exceed half the error gate).

Subset means: rng subset choices replicated exactly (trace-time constants).
For the fp8 stages the subset sums run on the PE as extra accumulation
matmuls (direct sum of member objects, or full-sum S minus negated
complement when that needs fewer terms); 1/|sel| folds into the g-half of
each weight bank. The final stage sums on the vector engine in bf16.

ELU: s*elu(x) = max(s*x, min(s*e^x, s) - s) with s*e^x from one Exp
activation (bias ln s). Sigmoids (CRN gates and the LSTM) use the tanh
form sigmoid(x) = (tanh(x/2)+1)/2 so every activation (Exp/Tanh/Copy)
stays in the exp_and_others table set -- no act-table reloads. The LSTM
tracks c2=2c, h2=2h with W_hh/Wvm pre-halved so no extra ops are needed.
Biases are folded per-channel only when any bias input is nonzero (the
general path); the all-zero case (checked host-side) uses wide ops.
"""

import functools
import itertools
import sys

import numpy as np

sys.path.insert(0, "/opt/trn_rl_repo")

import ml_dtypes  # noqa: E402

import concourse.bass as bass  # noqa: E402,F401
import concourse.mybir as mybir  # noqa: E402
import concourse.tile as tile  # noqa: E402
from concourse import bacc  # noqa: E402
from concourse.bass_utils import run_bass_kernel_spmd  # noqa: E402

BF = ml_dtypes.bfloat16
E4 = ml_dtypes.float8_e4m3
E3 = ml_dtypes.float8_e3m4
B, C, F, V, D = 128, 8, 16, 2048, 512
NCORES = 8
BS = B // NCORES      # 16 batch rows per core
J = BS * C            # 128 clip-level columns per core
T = F - 4             # 12 retained time slots
JV = BS * T           # 192 video-level columns per core

F32 = mybir.dt.float32
BF16 = mybir.dt.bfloat16
FP8E4 = mybir.dt.float8e4
FP8E3 = mybir.dt.float8e3
AF = mybir.ActivationFunctionType
OP = mybir.AluOpType
DR = mybir.MatmulPerfMode.DoubleRow

# ---------------------------------------------------------------- scales
SW = 2048.0           # e4m3 weight scale
SW3 = 64.0            # e3m4 weight scale (gW4)
S_APP = 16.0
S_MOT = 16.0
S_OBJ1 = 4.0          # objsT / S1 family
S_CONDM = 8.0
S_QP = 32.0           # condq
S_OBJ2 = 8.0          # objs2T / S2 family
S_OBJ3 = 16.0         # clipT family
S_VMP = 128.0

IMM_OBJS = S_OBJ1 / (SW * S_APP)
IMM_CONDM = S_CONDM / (SW * S_MOT)
INV_GX = 1.0 / (SW * S_MOT)
INV_1 = 1.0 / (SW * S_OBJ1)
IMM_1 = S_OBJ2 * INV_1
INV_2 = 1.0 / (SW * S_OBJ2)
IMM_2 = S_OBJ3 * INV_2
INV_3 = 1.0 / (SW * S_OBJ3)
INV_G4 = 1.0 / SW3
INV_HH = 1.0 / SW3    # whh/wvm ship as e3m4 x 64

# ---------------------------------------------------------------- subsets


def _subsets():
    """Replicate the reference's rng sequence exactly (trace-time constant)."""
    rng = np.random.RandomState(0)
    out = []
    for n in (F, F - 2, C, C - 2):
        sels = []
        for scale_id in range(1, n - 1):
            scale = n - scale_id
            rels = list(itertools.combinations(range(n), scale))
            idx = rng.choice(len(rels), min(1, len(rels)), replace=False)
            sels.append(list(rels[int(idx[0])]))
        out.append(sels)
    return out


SELS_M, SELS_Q, SELS_VM, SELS_VQ = _subsets()

# bias table layout (f32 [128, NBIAS]); constants first, general-path
# per-channel bias columns after.
COL_LN2 = 0     # ln(S_OBJ2)
COL_LN3 = 1     # ln(S_OBJ3 / 2)  (gated: t_z carries s/2)
COL_LNH = 2     # ln(1/2)
BOFF_A, BOFF_M, BOFF_Q, BOFF_VM, BOFF_G = 4, 8, 12, 16, 20
BOFF_1 = 36             # 14*4  (b1 + ln S_OBJ2 for exp; raw*s2 in BOFF_1L)
BOFF_1L = 92            # 14*4  (b1 * S_OBJ2)
BOFF_2 = 148            # 12*4  (b2 + ln(S_OBJ3/2))
BOFF_2L = 196           # 12*4  (b2 * S_OBJ3/2)
BOFF_G2 = 244           # 12*4  (gb2/2, tanh-form gate bias)
BOFF_3 = 292            # 6*4   (b3; exp bias, ln1=0)
BOFF_3L = 316           # 6*4   (b3)
BOFF_4 = 340            # 4*4   (b4 + ln(1/2))
BOFF_4L = 356           # 4*4   (b4/2)
BOFF_G4 = 372           # 4*4   (gb4/2)
NBIAS = 388


def _use_comp(n, sel):
    return (n - len(sel)) + 1 < len(sel)

# ---------------------------------------------------------------- device IR


def _gunits(n, sel, slicer, s_ap, neg_slicer):
    """Moving-operand list for the PE-side subset sum of `sel` over n objects:
    either the member slices, or [S] + negated complement slices."""
    if _use_comp(n, sel):
        in_set = set(sel)
        return [s_ap] + [neg_slicer(i) for i in range(n) if i not in in_set]
    return [slicer(f) for f in sel]


def _dr_group(nc, ps_m, wt_g, wt_c, units, cond_pairs, mslice, cond_first=True):
    """One PSUM accumulation group of DoubleRow matmuls: the cond pairs plus
    the g units. cond_first puts the cond matmuls first (their operands are
    usually ready early, keeping the PE busy across phase barriers); crn_vm
    uses cond_first=False because vmc lands late (after the LSTM chain).
    wt_g/wt_c: [128, 2, 2, 512] stationary halves; units/cond_pairs: lists of
    per-pair moving APs ([128, 2, N])."""
    total = len(units) * 2 + 2
    ops = []
    for u in units:
        for p in range(2):
            ops.append((wt_g[:, p, :, mslice], u[p]))
    cond_ops = [(wt_c[:, p, :, mslice], cond_pairs[p]) for p in range(2)]
    ops = cond_ops + ops if cond_first else ops + cond_ops
    for k, (w, x) in enumerate(ops):
        nc.tensor.matmul(ps_m, w, x, start=(k == 0), stop=(k == total - 1),
                         perf_mode=DR)


def _pairs(ap4):
    """[128, 2, 2, N] AP -> per-pair [128, 2, N] moving APs."""
    return [ap4[:, 0, :, :], ap4[:, 1, :, :]]


@functools.lru_cache(maxsize=2)
def _program(biasfree=True, debug=False):
    nc = bacc.Bacc("TRN2", target_bir_lowering=False, debug=False,
                   num_devices=NCORES)

    app_d = nc.dram_tensor("app", [128, 4, 2, 8, 2, 256], FP8E4, kind="ExternalInput")
    mot_d = nc.dram_tensor("mot", [128, 8, 2, J], FP8E4, kind="ExternalInput")
    q_d = nc.dram_tensor("q", [128, 4, BS], BF16, kind="ExternalInput")
    wa_d = nc.dram_tensor("wa", [128, 8, 2, 512], FP8E4, kind="ExternalInput")
    wm_d = nc.dram_tensor("wm", [128, 8, 2, 512], FP8E4, kind="ExternalInput")
    wq_d = nc.dram_tensor("wq", [128, 4, 512], BF16, kind="ExternalInput")
    wvm_d = nc.dram_tensor("wvm", [128, 4, 512], FP8E3, kind="ExternalInput")
    wih_d = nc.dram_tensor("wih", [128, 16, 8, 2, 128], FP8E4, kind="ExternalInput")
    whh_d = nc.dram_tensor("whh", [128, 4, 2048], FP8E3, kind="ExternalInput")
    w1_d = nc.dram_tensor("w1", [128, 14, 2, 2, 2, 512], FP8E4, kind="ExternalInput")
    w2_d = nc.dram_tensor("w2", [128, 12, 2, 2, 2, 512], FP8E4, kind="ExternalInput")
    w2g_d = nc.dram_tensor("w2g", [128, 12, 2, 2, 2, 512], FP8E4, kind="ExternalInput")
    w3_d = nc.dram_tensor("w3", [128, 6, 2, 2, 2, 512], FP8E4, kind="ExternalInput")
    w4_d = nc.dram_tensor("w4", [128, 4, 8, 512], BF16, kind="ExternalInput")
    w4g_d = nc.dram_tensor("w4g", [128, 4, 8, 512], FP8E3, kind="ExternalInput")
    bias_d = nc.dram_tensor("bias", [128, NBIAS], F32, kind="ExternalInput")
    out_d = nc.dram_tensor("out", [128, 4, 4 * JV], BF16, kind="ExternalOutput")

    nc._phases = []

    def _mark(name):
        nc._phases.append((name, int(nc.get_next_instruction_name()[2:])))

    with tile.TileContext(nc) as tc:
        # Pools form a strict stack (release order = reverse of allocation).
        perm = tc.alloc_tile_pool(name="perm", bufs=1)
        tpool = tc.alloc_tile_pool(name="tmp", bufs=4)
        stream = tc.alloc_tile_pool(name="stream", bufs=4)
        p5 = tc.alloc_tile_pool(name="p5", bufs=1)        # clipT
        p4 = tc.alloc_tile_pool(name="p4", bufs=1)        # objs2T (+neg, S2)
        p3 = tc.alloc_tile_pool(name="p3", bufs=1)        # objsT (+neg, S1), condm
        p0 = tc.alloc_tile_pool(name="p0", bufs=1)        # early consts
        pp_early = tc.alloc_tile_pool(name="ps_early", bufs=1, space="PSUM")

        _mark("consts")
        bias = perm.tile([128, NBIAS], F32, name="bias")
        nc.sync.dma_start(bias, bias_d[:])

        def bap(off):
            return bias[:, off:off + 1]

        motT = p0.tile([128, 8, 2, J], FP8E4, name="motT")
        nc.sync.dma_start(motT, mot_d[:])
        qT = p0.tile([128, 4, BS], BF16, name="qT")
        nc.sync.dma_start(qT, q_d[:])
        wqt = p0.tile([128, 4, 512], BF16, name="wqt")
        nc.sync.dma_start(wqt, wq_d[:])

        _mark("qproj_condm")
        # ---------------- q_proj (bf16) -> qp [128, 4, BS]
        psq = pp_early.tile([128, 4, BS], F32, tag="psq", name="psq")
        for m in range(4):
            for kc in range(4):
                nc.tensor.matmul(psq[:, m, :], wqt[:, kc, m * 128:(m + 1) * 128],
                                 qT[:, kc, :], start=(kc == 0), stop=(kc == 3))
        qp = perm.tile([128, 4, BS], BF16, name="qp")
        if biasfree:
            nc.vector.tensor_copy(qp, psq)
        else:
            for m in range(4):
                nc.vector.tensor_scalar_add(qp[:, m, :], psq[:, m, :],
                                            bap(BOFF_Q + m))

        # ---------------- mot_proj (DR) -> condm [128, 2, 2, J] e4m3
        wmt = p0.tile([128, 8, 2, 512], FP8E4, name="wmt")
        nc.sync.dma_start(wmt, wm_d[:])
        pscm = pp_early.tile([128, 4, J], F32, tag="pscm", name="pscm")
        for m in range(4):
            for p in range(8):
                nc.tensor.matmul(pscm[:, m, :], wmt[:, p, :, m * 128:(m + 1) * 128],
                                 motT[:, p, :, :], start=(p == 0), stop=(p == 7),
                                 perf_mode=DR)
        condm = p3.tile([128, 2, 2, J], FP8E4, name="condm")
        condm_w = condm.rearrange("p a b j -> p (a b) j")
        if biasfree:
            nc.vector.tensor_scalar_mul(condm_w, pscm, IMM_CONDM)
        else:
            for m in range(4):
                nc.vector.tensor_scalar(condm_w[:, m, :], pscm[:, m, :],
                                        IMM_CONDM, bap(BOFF_M + m),
                                        OP.mult, OP.add)

        # cond broadcasts: condq e4m3 (x S_QP), qvc bf16
        condq = perm.tile([128, 2, 2, BS, C], FP8E4, name="condq")
        nc.vector.tensor_scalar_mul(
            condq.rearrange("p a b s c -> p (a b) s c"),
            qp[:, :, :, None].to_broadcast([128, 4, BS, C]), S_QP)
        qvc = perm.tile([128, 4, BS, T], BF16, name="qvc")
        nc.vector.tensor_copy(qvc, qp[:, :, :, None].to_broadcast([128, 4, BS, T]))
        qvc_v = qvc.rearrange("p d b t -> p d (b t)")
        pp_early.release()

        _mark("stageA")
        # ---------------- stage A: app_proj (DR) -> objsT/neg [128,2,2,F,J]
        p2 = tc.alloc_tile_pool(name="p2", bufs=1)
        apps = tc.alloc_tile_pool(name="apps", bufs=3)
        pp_a = tc.alloc_tile_pool(name="ps_a", bufs=2, space="PSUM")
        wat = p2.tile([128, 8, 2, 512], FP8E4, name="wat")
        nc.sync.dma_start(wat, wa_d[:])
        objsT = p3.tile([128, 2, 2, F, J], FP8E4, name="objsT")
        nobjsT = p3.tile([128, 2, 2, F, J], FP8E4, name="nobjsT")
        for cc in range(4):
            xc = apps.tile([128, 2, 8, 2, 256], FP8E4, tag="app", name="xc")
            nc.sync.dma_start(xc, app_d[:, cc, :, :, :, :])
            for m in range(4):
                ps_a = pp_a.tile([128, 512], F32, tag="psA", name="ps_a")
                for h in range(2):
                    for p in range(8):
                        nc.tensor.matmul(ps_a[:, h * 256:(h + 1) * 256],
                                         wat[:, p, :, m * 128:(m + 1) * 128],
                                         xc[:, h, p, :, :],
                                         start=(p == 0), stop=(p == 7),
                                         perf_mode=DR)
                dst = objsT[:, m // 2, m % 2, cc * 4:(cc + 1) * 4, :]
                dst = dst.rearrange("p f j -> p (f j)")
                ndst = nobjsT[:, m // 2, m % 2, cc * 4:(cc + 1) * 4, :]
                ndst = ndst.rearrange("p f j -> p (f j)")
                if biasfree:
                    nc.vector.tensor_scalar_mul(dst, ps_a, IMM_OBJS)
                else:
                    nc.vector.tensor_scalar(dst, ps_a, IMM_OBJS,
                                            bap(BOFF_A + m), OP.mult, OP.add)
                nc.scalar.mul(ndst, dst, -1.0)
        pp_a.release()
        apps.release()
        p2.release()

        # S1 = sum_f objsT (two-accumulator bf16 chain, final e4m3)
        s1 = p3.tile([128, 2, 2, J], FP8E4, name="s1")
        s1a = p3.tile([128, 2, 2, J], BF16, name="s1a")
        s1b = p3.tile([128, 2, 2, J], BF16, name="s1b")
        nc.vector.tensor_add(s1a, objsT[:, :, :, 0, :], objsT[:, :, :, 1, :])
        nc.vector.tensor_add(s1b, objsT[:, :, :, 2, :], objsT[:, :, :, 3, :])
        for f in range(4, F):
            t = s1a if f % 2 == 0 else s1b
            nc.vector.tensor_add(t, t, objsT[:, :, :, f, :])
        nc.vector.tensor_add(s1, s1a, s1b)

        _mark("crn_m")
        # ---------------- crn_m: objsT -> objs2T [128, 2, 2, 14, J]
        pp_crn = tc.alloc_tile_pool(name="ps_crn", bufs=2, space="PSUM")
        objs2T = p4.tile([128, 2, 2, 14, J], FP8E4, name="objs2T")
        nobjs2T = p4.tile([128, 2, 2, 14, J], FP8E4, name="nobjs2T")
        s2a = p4.tile([128, 2, 2, J], BF16, name="s2a")

        def obj1(f):
            return _pairs(objsT[:, :, :, f, :])

        def nobj1(f):
            return _pairs(nobjsT[:, :, :, f, :])

        # complement scales last so S1/neg have time to complete
        order_m = ([i for i, s in enumerate(SELS_M) if not _use_comp(F, s)]
                   + [i for i, s in enumerate(SELS_M) if _use_comp(F, s)])
        cond_m_pairs = _pairs(condm)
        for oi, si in enumerate(order_m):
            sel = SELS_M[si]
            w1t = stream.tile([128, 2, 2, 2, 512], FP8E4, tag="crnw", name="w1t", bufs=10)
            nc.sync.dma_start(w1t, w1_d[:, si, :, :, :, :])
            if _use_comp(F, sel):
                in_set = set(sel)
                units = [_pairs(s1)] + [nobj1(i) for i in range(F)
                                        if i not in in_set]
            else:
                units = [obj1(f) for f in sel]
            ps = pp_crn.tile([128, 4, J], F32, tag="psM", name="ps_m1", bufs=3)
            for m in range(4):
                _dr_group(nc, ps[:, m, :], w1t[:, 0], w1t[:, 1], units,
                          cond_m_pairs, slice(m * 128, (m + 1) * 128))
            # epilogue: objs2T[si] = S_OBJ2 * elu(inv1 * ps + b)
            t_e = tpool.tile([128, 4, J], BF16, tag="t_exp", name="t_e", bufs=2)
            t_m = tpool.tile([128, 4, J], BF16, tag="t_min", name="t_m", bufs=2)
            dst = objs2T[:, :, :, si, :].rearrange("p a b j -> p (a b) j")
            ndst = nobjs2T[:, :, :, si, :].rearrange("p a b j -> p (a b) j")
            if biasfree:
                nc.scalar.activation(t_e, ps, AF.Exp, bias=bap(COL_LN2),
                                     scale=INV_1)
                nc.vector.tensor_scalar(t_m, t_e, S_OBJ2, -S_OBJ2, OP.min, OP.add)
                nc.vector.scalar_tensor_tensor(dst, ps, IMM_1, t_m,
                                               OP.mult, OP.max)
            else:
                for m in range(4):
                    nc.scalar.activation(t_e[:, m, :], ps[:, m, :], AF.Exp,
                                         bias=bap(BOFF_1 + si * 4 + m),
                                         scale=INV_1)
                nc.vector.tensor_scalar(t_m, t_e, S_OBJ2, -S_OBJ2, OP.min, OP.add)
                for m in range(4):
                    lin = tpool.tile([128, J], F32, tag="lin", name="lin")
                    nc.vector.tensor_scalar(lin, ps[:, m, :], IMM_1,
                                            bap(BOFF_1L + si * 4 + m),
                                            OP.mult, OP.add)
                    nc.vector.tensor_tensor(dst[:, m, :], lin, t_m[:, m, :], OP.max)
            nc.scalar.mul(ndst, dst, -1.0)
            # incremental S2
            s2src = objs2T[:, :, :, si, :]
            if oi == 0:
                nc.vector.tensor_copy(s2a, s2src)
            else:
                nc.vector.tensor_add(s2a, s2a, s2src)
        s2 = p4.tile([128, 2, 2, J], FP8E4, name="s2")
        nc.vector.tensor_copy(s2, s2a)

        _mark("gatesx")
        # ---------------- LSTM x-gates: gx = inv * (W_ih @ motT)  (DR)
        wihs = tc.alloc_tile_pool(name="wihs", bufs=10)
        p1 = tc.alloc_tile_pool(name="p1", bufs=1)
        ppx = tc.alloc_tile_pool(name="ps_x", bufs=2, space="PSUM")
        whht = p1.tile([128, 4, 2048], FP8E3, name="whht")
        nc.sync.dma_start(whht, whh_d[:])
        wvmt = p1.tile([128, 4, 512], FP8E3, name="wvmt")
        nc.sync.dma_start(wvmt, wvm_d[:])
        gx = p1.tile([128, 16, J], F32, name="gx")
        for mi in range(16):
            wih_t = wihs.tile([128, 8, 2, 128], FP8E4, tag="wih", name="wih_t")
            nc.sync.dma_start(wih_t, wih_d[:, mi, :, :, :])
            psx = ppx.tile([128, J], F32, tag="psx", name="psx")
            for p in range(8):
                nc.tensor.matmul(psx, wih_t[:, p, :, :], motT[:, p, :, :],
                                 start=(p == 0), stop=(p == 7), perf_mode=DR)
            if biasfree:
                nc.vector.tensor_scalar_mul(gx[:, mi, :], psx, INV_GX)
            else:
                nc.vector.tensor_scalar(gx[:, mi, :], psx, INV_GX,
                                        bap(BOFF_G + mi), OP.mult, OP.add)
        ppx.release()
        pp_r = tc.alloc_tile_pool(name="ps_r", bufs=2, space="PSUM")
        gxr = gx.rearrange("p m (b c) -> p m c b", c=C)

        _mark("lstm")
        # ---------------- LSTM recurrence, tanh-only form (one act table):
        # sigmoid(x) = (tanh(x/2)+1)/2. Track c2 = 2c and h2 = 2h; the /2 of
        # each sigmoid folds into stt imms and W_hh/Wvm are pre-halved on the
        # host so psr = W_hh @ h exactly.
        h_prev = None
        c_prev = None
        for t in range(C):
            xg = gxr[:, :, t, :]
            if t == 0:
                gates = xg
            else:
                psr = pp_r.tile([128, 16, BS], F32, tag="psr", name="psr")
                for mi in range(16):
                    for kc in range(4):
                        nc.tensor.matmul(psr[:, mi, :],
                                         whht[:, kc, mi * 128:(mi + 1) * 128],
                                         h_prev[:, kc, :],
                                         start=(kc == 0), stop=(kc == 3))
                gates = tpool.tile([128, 16, BS], F32, tag="lstm_g", name="lstm_g")
                nc.vector.scalar_tensor_tensor(gates, psr, INV_HH, xg,
                                               OP.mult, OP.add)
            t_if = tpool.tile([128, 8, BS], F32, tag="dif", name="t_if")
            nc.scalar.activation(t_if, gates[:, 0:8, :], AF.Tanh, scale=0.5)
            tan_g = tpool.tile([128, 4, BS], F32, tag="tg", name="tan_g")
            nc.scalar.activation(tan_g, gates[:, 8:12, :], AF.Tanh)
            t_o = tpool.tile([128, 4, BS], F32, tag="do", name="t_o")
            nc.scalar.activation(t_o, gates[:, 12:16, :], AF.Tanh, scale=0.5)
            # ig2 = (tanh_i+1)*tan_g = 2*sig_i*tan_g
            ig2 = tpool.tile([128, 4, BS], F32, tag="ig", name="ig2", bufs=2)
            nc.vector.scalar_tensor_tensor(ig2, t_if[:, 0:4, :], 1.0, tan_g,
                                           OP.add, OP.mult)
            if t == 0:
                c2_t = ig2
            else:
                # fc2 = (tanh_f+1)*c2_prev = 4*sig_f*c ; c2 = fc2/2 + ig2
                c2_t = tpool.tile([128, 4, BS], F32, tag="c_t", name="c2_t", bufs=2)
                fc2 = tpool.tile([128, 4, BS], F32, tag="fc", name="fc2")
                nc.vector.scalar_tensor_tensor(fc2, t_if[:, 4:8, :], 1.0,
                                               c_prev, OP.add, OP.mult)
                nc.vector.scalar_tensor_tensor(c2_t, fc2, 0.5, ig2,
                                               OP.mult, OP.add)
            tan_c = tpool.tile([128, 4, BS], F32, tag="tanc", name="tan_c")
            nc.scalar.activation(tan_c, c2_t, AF.Tanh, scale=0.5)
            # h2 = (tanh_o+1)*tanh(c) = 2h
            h_t = tpool.tile([128, 4, BS], BF16, tag="h_t", name="h2_t", bufs=2)
            nc.vector.scalar_tensor_tensor(h_t, t_o, 1.0, tan_c,
                                           OP.add, OP.mult)
            h_prev, c_prev = h_t, c2_t

        # vm_proj (bf16) -> vmc [128, 2, 2, BS, T] e4m3 (x S_VMP)
        psv = pp_r.tile([128, 4, BS], F32, tag="psv", name="psv", bufs=1)
        for m in range(4):
            for kc in range(4):
                nc.tensor.matmul(psv[:, m, :], wvmt[:, kc, m * 128:(m + 1) * 128],
                                 h_prev[:, kc, :], start=(kc == 0), stop=(kc == 3))
        vmp = p1.tile([128, 4, BS], FP8E4, name="vmp")
        if biasfree:
            nc.vector.tensor_scalar_mul(vmp, psv, S_VMP * INV_HH)
        else:
            for m in range(4):
                nc.vector.tensor_scalar(vmp[:, m, :], psv[:, m, :], S_VMP * INV_HH,
                                        bap(BOFF_VM + m), OP.mult, OP.add)
        vmc = perm.tile([128, 2, 2, BS, T], FP8E4, name="vmc")
        nc.vector.tensor_copy(
            vmc.rearrange("p a b s t -> p (a b) s t"),
            vmp[:, :, :, None].to_broadcast([128, 4, BS, T]))
        pp_r.release()
        p1.release()
        wihs.release()

        _mark("crn_q")
        # ---------------- crn_q (gated): objs2T -> clipT [128,2,2,C,BS,T]
        clipT = p5.tile([128, 2, 2, C, BS, T], FP8E4, name="clipT")

        def obj2(s):
            return _pairs(objs2T[:, :, :, s, :])

        def nobj2(s):
            return _pairs(nobjs2T[:, :, :, s, :])

        order_q = ([i for i, s in enumerate(SELS_Q) if not _use_comp(F - 2, s)]
                   + [i for i, s in enumerate(SELS_Q) if _use_comp(F - 2, s)])
        condq_pairs = _pairs(condq.rearrange("p a b s c -> p a b (s c)"))
        for si in order_q:
            sel = SELS_Q[si]
            w2t = stream.tile([128, 2, 2, 2, 512], FP8E4, tag="crnw", name="w2t", bufs=10)
            nc.sync.dma_start(w2t, w2_d[:, si, :, :, :, :])
            w2gt = stream.tile([128, 2, 2, 2, 512], FP8E4, tag="crnw", name="w2gt", bufs=10)
            nc.sync.dma_start(w2gt, w2g_d[:, si, :, :, :, :])
            if _use_comp(F - 2, sel):
                in_set = set(sel)
                units = [_pairs(s2)] + [nobj2(i) for i in range(F - 2)
                                        if i not in in_set]
            else:
                units = [obj2(s) for s in sel]
            ps_m = pp_crn.tile([128, 4, J], F32, tag="psM", name="ps_q1", bufs=3)
            ps_g = pp_crn.tile([128, 4, J], F32, tag="psG", name="ps_q2")
            for m in range(4):
                _dr_group(nc, ps_m[:, m, :], w2t[:, 0], w2t[:, 1], units,
                          condq_pairs, slice(m * 128, (m + 1) * 128))
            for m in range(4):
                _dr_group(nc, ps_g[:, m, :], w2gt[:, 0], w2gt[:, 1], units,
                          condq_pairs, slice(m * 128, (m + 1) * 128))
            # gated epilogue, tanh form: t_z carries S_OBJ3/2 * elu;
            # out = (tanh(gate/2)+1) * t_z = S_OBJ3 * elu * sigmoid(gate)
            t_e = tpool.tile([128, 4, J], BF16, tag="t_exp", name="t_e", bufs=2)
            t_m = tpool.tile([128, 4, J], BF16, tag="t_min", name="t_m", bufs=2)
            t_z = tpool.tile([128, 4, J], BF16, tag="t_z", name="t_z", bufs=2)
            t_d = tpool.tile([128, 4, J], BF16, tag="t_d", name="t_d", bufs=2)
            h3 = S_OBJ3 / 2
            if biasfree:
                nc.scalar.activation(t_e, ps_m, AF.Exp, bias=bap(COL_LN3),
                                     scale=INV_2)
                nc.vector.tensor_scalar(t_m, t_e, h3, -h3, OP.min, OP.add)
                nc.vector.scalar_tensor_tensor(t_z, ps_m, IMM_2 / 2, t_m,
                                               OP.mult, OP.max)
                nc.scalar.activation(t_d, ps_g, AF.Tanh, scale=INV_2 / 2)
            else:
                for m in range(4):
                    nc.scalar.activation(t_e[:, m, :], ps_m[:, m, :], AF.Exp,
                                         bias=bap(BOFF_2 + si * 4 + m),
                                         scale=INV_2)
                    nc.scalar.activation(t_d[:, m, :], ps_g[:, m, :], AF.Tanh,
                                         bias=bap(BOFF_G2 + si * 4 + m),
                                         scale=INV_2 / 2)
                nc.vector.tensor_scalar(t_m, t_e, h3, -h3, OP.min, OP.add)
                for m in range(4):
                    lin = tpool.tile([128, J], F32, tag="lin", name="lin")
                    nc.vector.tensor_scalar(lin, ps_m[:, m, :], IMM_2 / 2,
                                            bap(BOFF_2L + si * 4 + m),
                                            OP.mult, OP.add)
                    nc.vector.tensor_tensor(t_z[:, m, :], lin, t_m[:, m, :], OP.max)
            # dst view: cols j=(b c) -> clipT[:, :, :, c, b, si].
            # (t_d + 1) via a 3D-out tensor_scalar first: ScalarTensorTensor
            # outputs must be <= 3D and the clipT view is 4D.
            nc.vector.tensor_scalar_add(t_d, t_d, 1.0)
            wide = clipT[:, :, :, :, :, si].rearrange("p a b c s -> p (a b) s c")
            nc.vector.tensor_tensor(wide, t_d.rearrange("p d (s c) -> p d s c", c=C),
                                    t_z.rearrange("p d (s c) -> p d s c", c=C),
                                    OP.mult)
        pp_crn.release()
        p0.release()
        p3.release()
        p4.release()

        _mark("crn_vm")
        # ---------------- crn_vm (ungated, direct sums): clipT -> objs4T bf16
        pp_v = tc.alloc_tile_pool(name="ps_v", bufs=1, space="PSUM")
        objs4T = perm.tile([128, 4, 6, JV], BF16, name="objs4T")

        def clip_pairs(c):
            ap = clipT[:, :, :, c, :, :]
            return _pairs(ap.rearrange("p a b s t -> p a b (s t)"))

        vmc_pairs = _pairs(vmc.rearrange("p a b s t -> p a b (s t)"))
        # incremental S4 accumulator (bf16) so crn_vq's complement scales can
        # start right after the last crn_vm epilogue
        s4 = perm.tile([128, 4, JV], BF16, name="s4")
        for si, sel in enumerate(SELS_VM):
            w3t = stream.tile([128, 2, 2, 2, 512], FP8E4, tag="crnw", name="w3t", bufs=10)
            nc.sync.dma_start(w3t, w3_d[:, si, :, :, :, :])
            units = [clip_pairs(c) for c in sel]
            # alternate tag pairs so 4 scale-epilogues can be in flight
            # (psV2/3 are otherwise idle until crn_vq)
            t0, t1 = ("psV0", "psV1") if si % 2 == 0 else ("psV2", "psV3")
            ps0 = pp_v.tile([128, 2, JV], F32, tag=t0, name="ps_vm0", bufs=2)
            ps1 = pp_v.tile([128, 2, JV], F32, tag=t1, name="ps_vm1", bufs=2)
            ps_list = [ps0[:, 0, :], ps0[:, 1, :], ps1[:, 0, :], ps1[:, 1, :]]
            for m in range(4):
                _dr_group(nc, ps_list[m], w3t[:, 0], w3t[:, 1], units,
                          vmc_pairs, slice(m * 128, (m + 1) * 128))
            for half, psh in enumerate((ps0, ps1)):
                t_e = tpool.tile([128, 2, JV], BF16, tag="t_expv", name="t_ev", bufs=2)
                t_m = tpool.tile([128, 2, JV], BF16, tag="t_minv", name="t_mv", bufs=2)
                dst = objs4T[:, half * 2:(half + 1) * 2, si, :]
                if biasfree:
                    nc.scalar.activation(t_e, psh, AF.Exp, scale=INV_3)
                    nc.vector.tensor_scalar(t_m, t_e, 1.0, -1.0, OP.min, OP.add)
                    nc.vector.scalar_tensor_tensor(dst, psh, INV_3, t_m,
                                                   OP.mult, OP.max)
                else:
                    for mm in range(2):
                        m = half * 2 + mm
                        nc.scalar.activation(t_e[:, mm, :], psh[:, mm, :], AF.Exp,
                                             bias=bap(BOFF_3 + si * 4 + m),
                                             scale=INV_3)
                        nc.vector.tensor_scalar(t_m[:, mm, :], t_e[:, mm, :],
                                                1.0, -1.0, OP.min, OP.add)
                        lin = tpool.tile([128, JV], F32, tag="linv", name="linv")
                        nc.vector.tensor_scalar(lin, psh[:, mm, :], INV_3,
                                                bap(BOFF_3L + si * 4 + m),
                                                OP.mult, OP.add)
                        nc.vector.tensor_tensor(dst[:, mm, :], lin, t_m[:, mm, :],
                                                OP.max)
            # incremental S4
            s4src = objs4T[:, :, si, :]
            if si == 0:
                nc.vector.tensor_copy(s4, s4src)
            else:
                nc.vector.tensor_add(s4, s4, s4src)

        _mark("crn_vq")
        # ---------------- crn_vq (bf16, gated): objs4T -> out
        def o4_slice(s):
            return objs4T[:, :, s, :]

        gpool = tc.alloc_tile_pool(name="gpool", bufs=4)
        # direct-sum scales first: they don't need s4
        order_vq = ([i for i, s in enumerate(SELS_VQ)
                     if not (C - 2 - len(s)) + 1 < len(s)]
                    + [i for i, s in enumerate(SELS_VQ)
                       if (C - 2 - len(s)) + 1 < len(s)])
        for si in order_vq:
            sel = SELS_VQ[si]
            w4t = stream.tile([128, 8, 512], BF16, tag="crnw4", name="w4t", bufs=2)
            nc.sync.dma_start(w4t, w4_d[:, si, :, :])
            w4gt = stream.tile([128, 8, 512], FP8E3, tag="crnw4g", name="w4gt", bufs=2)
            nc.sync.dma_start(w4gt, w4g_d[:, si, :, :])
            # g = subset sum (bf16 DVE, complement vs direct)
            in_set = set(sel)
            comp = [i for i in range(C - 2) if i not in in_set]
            if len(comp) + 1 < len(sel):
                g = gpool.tile([128, 4, JV], BF16, tag="g4", name="g4")
                nc.vector.tensor_sub(g, s4, o4_slice(comp[0]))
                for i in comp[1:]:
                    nc.vector.tensor_sub(g, g, o4_slice(i))
            elif len(sel) == 1:
                g = o4_slice(sel[0])
            else:
                g = gpool.tile([128, 4, JV], BF16, tag="g4", name="g4")
                nc.vector.tensor_add(g, o4_slice(sel[0]), o4_slice(sel[1]))
                for i in sel[2:]:
                    nc.vector.tensor_add(g, g, o4_slice(i))
            ps0 = pp_v.tile([128, 2, JV], F32, tag="psV0", name="ps_vq0", bufs=2)
            ps1 = pp_v.tile([128, 2, JV], F32, tag="psV1", name="ps_vq1", bufs=2)
            pg0 = pp_v.tile([128, 2, JV], F32, tag="psV2", name="ps_vq2", bufs=2)
            pg1 = pp_v.tile([128, 2, JV], F32, tag="psV3", name="ps_vq3", bufs=2)
            ps_list = [ps0[:, 0, :], ps0[:, 1, :], ps1[:, 0, :], ps1[:, 1, :]]
            pg_list = [pg0[:, 0, :], pg0[:, 1, :], pg1[:, 0, :], pg1[:, 1, :]]
            for m in range(4):
                msl = slice(m * 128, (m + 1) * 128)
                for kc in range(4):
                    nc.tensor.matmul(ps_list[m], w4t[:, 4 + kc, msl],
                                     qvc_v[:, kc, :], start=(kc == 0), stop=False)
                for kc in range(4):
                    nc.tensor.matmul(ps_list[m], w4t[:, kc, msl], g[:, kc, :],
                                     start=False, stop=(kc == 3))
            for m in range(4):
                msl = slice(m * 128, (m + 1) * 128)
                for kc in range(4):
                    nc.tensor.matmul(pg_list[m], w4gt[:, 4 + kc, msl],
                                     qvc_v[:, kc, :], start=(kc == 0), stop=False)
                for kc in range(4):
                    nc.tensor.matmul(pg_list[m], w4gt[:, kc, msl], g[:, kc, :],
                                     start=False, stop=(kc == 3))
            ot = tpool.tile([128, 4, JV], BF16, tag="ot", name="ot4", bufs=2)
            for half, (psh, pgh) in enumerate(((ps0, pg0), (ps1, pg1))):
                t_e = tpool.tile([128, 2, JV], BF16, tag="t_expv", name="t_ev", bufs=2)
                t_m = tpool.tile([128, 2, JV], BF16, tag="t_minv", name="t_mv", bufs=2)
                t_z = tpool.tile([128, 2, JV], BF16, tag="t_zv", name="t_zv", bufs=2)
                t_d = tpool.tile([128, 2, JV], BF16, tag="t_dv", name="t_dv", bufs=2)
                oth = ot[:, half * 2:(half + 1) * 2, :]
                if biasfree:
                    nc.scalar.activation(t_e, psh, AF.Exp, bias=bap(COL_LNH))
                    nc.vector.tensor_scalar(t_m, t_e, 0.5, -0.5, OP.min, OP.add)
                    nc.vector.scalar_tensor_tensor(t_z, psh, 0.5, t_m,
                                                   OP.mult, OP.max)
                    nc.scalar.activation(t_d, pgh, AF.Tanh, scale=INV_G4 / 2)
                    nc.vector.scalar_tensor_tensor(oth, t_d, 1.0, t_z,
                                                   OP.add, OP.mult)
                else:
                    for mm in range(2):
                        m = half * 2 + mm
                        nc.scalar.activation(t_e[:, mm, :], psh[:, mm, :], AF.Exp,
                                             bias=bap(BOFF_4 + si * 4 + m))
                        nc.scalar.activation(t_d[:, mm, :], pgh[:, mm, :],
                                             AF.Tanh,
                                             bias=bap(BOFF_G4 + si * 4 + m),
                                             scale=INV_G4 / 2)
                        nc.vector.tensor_scalar(t_m[:, mm, :], t_e[:, mm, :],
                                                0.5, -0.5, OP.min, OP.add)
                        lin = tpool.tile([128, JV], F32, tag="linv", name="linv")
                        nc.vector.tensor_scalar(lin, psh[:, mm, :], 0.5,
                                                bap(BOFF_4L + si * 4 + m),
                                                OP.mult, OP.add)
                        nc.vector.tensor_tensor(t_z[:, mm, :], lin, t_m[:, mm, :],
                                                OP.max)
                        nc.vector.scalar_tensor_tensor(oth[:, mm, :],
                                                       t_d[:, mm, :], 1.0,
                                                       t_z[:, mm, :],
                                                       OP.add, OP.mult)
            nc.sync.dma_start(out_d[:, si, :], ot.rearrange("p d j -> p (d j)"))

        for pool in (gpool, pp_v, p5, stream, tpool, perm):
            pool.release()

    nc.compile()
    return nc


# ---------------------------------------------------------------- host side


def _kxm_pairs(w_t, scale, dt):
    """[K, M] f32 -> [128, K//256, 2, M] scaled/clipped fp8 (pair-grouped)."""
    K, M = w_t.shape
    lim = 240.0 if dt is E4 else 15.5
    w = np.clip(np.asarray(w_t, np.float32) * scale, -lim, lim)
    return np.ascontiguousarray(
        w.reshape(K // 256, 2, 128, M).transpose(2, 0, 1, 3)).astype(dt)


def _bank_dr(Ws, sels, s_cond_ratio):
    """CRN bank -> [128, S, 2, 2, 2, 512] e4m3: halves (g/|sel|, c*ratio)."""
    per = []
    for si, sel in enumerate(sels):
        s_id = si + 1
        hg = np.asarray(Ws[s_id][:, :D], np.float32).T / len(sel) * SW
        hc = np.asarray(Ws[s_id][:, D:], np.float32).T * (SW * s_cond_ratio)
        h = np.stack([hg, hc])  # [2, 512, 512]
        h = np.clip(h, -240, 240)
        per.append(h.reshape(2, 2, 2, 128, 512).transpose(3, 0, 1, 2, 4))
    return np.ascontiguousarray(np.stack(per, axis=1)).astype(E4)


def _bank_bf16(Ws, sels, dt=BF, scale=1.0):
    """Stage-4 bank -> [128, S, 8, 512] (halves g/|sel|, c as 4+4 k-chunks)."""
    lim = {BF: 3e38, E3: 15.5, E4: 240.0}[dt]
    per = []
    for si, sel in enumerate(sels):
        s_id = si + 1
        hg = np.asarray(Ws[s_id][:, :D], np.float32).T / len(sel) * scale
        hc = np.asarray(Ws[s_id][:, D:], np.float32).T * scale
        h = np.concatenate([hg, hc], axis=0)  # [1024, 512]
        h = np.clip(h, -lim, lim)
        per.append(h.reshape(8, 128, 512).transpose(1, 0, 2))
    return np.ascontiguousarray(np.stack(per, axis=1)).astype(dt)


def _kxm_e3(w_t, kchunks):
    K, M = w_t.shape
    w = np.clip(np.asarray(w_t, np.float32) * SW3, -15.5, 15.5)
    return np.ascontiguousarray(
        w.reshape(kchunks, 128, M).transpose(1, 0, 2)).astype(E3)


def _to_kxm_bf16(w_t, kchunks):
    K, M = w_t.shape
    return np.ascontiguousarray(
        np.asarray(w_t, np.float32).reshape(kchunks, 128, M)
        .transpose(1, 0, 2)).astype(BF)


def _vec_to_pm(v, chunks):
    return np.ascontiguousarray(np.asarray(v, np.float32).reshape(chunks, 128).T)


def _prep_weights(inputs):
    w = {}
    w["wa"] = _kxm_pairs(np.asarray(inputs["Wa"], np.float32).T, SW, E4)
    w["wm"] = _kxm_pairs(np.asarray(inputs["Wm"], np.float32).T, SW, E4)
    w["wq"] = _to_kxm_bf16(np.asarray(inputs["Wq"], np.float32).T, 4)
    # Wvm/W_hh halved (device tracks h2 = 2h), shipped e3m4 x 64
    w["wvm"] = _kxm_e3(np.asarray(inputs["Wvm"], np.float32).T / 2, 4)
    wih = _kxm_pairs(np.asarray(inputs["W_ih"], np.float32).T, SW, E4)
    # [128, 8, 2, 2048] -> [128, mi 16, pair 8, 2, 128]
    w["wih"] = np.ascontiguousarray(
        wih.reshape(128, 8, 2, 16, 128).transpose(0, 3, 1, 2, 4))
    w["whh"] = _kxm_e3(np.asarray(inputs["W_hh"], np.float32).T / 2, 4)
    w["w1"] = _bank_dr(np.asarray(inputs["W1"], np.float32), SELS_M,
                       S_OBJ1 / S_CONDM)
    w["w2"] = _bank_dr(np.asarray(inputs["W2"], np.float32), SELS_Q,
                       S_OBJ2 / S_QP)
    w["w2g"] = _bank_dr(np.asarray(inputs["gW2"], np.float32), SELS_Q,
                        S_OBJ2 / S_QP)
    w["w3"] = _bank_dr(np.asarray(inputs["W3"], np.float32), SELS_VM,
                       S_OBJ3 / S_VMP)
    w["w4"] = _bank_bf16(np.asarray(inputs["W4"], np.float32), SELS_VQ)
    w["w4g"] = _bank_bf16(np.asarray(inputs["gW4"], np.float32), SELS_VQ,
                          dt=E3, scale=SW3)

    bias = np.zeros((128, NBIAS), np.float32)
    bias[:, COL_LN2] = np.log(S_OBJ2)
    bias[:, COL_LN3] = np.log(S_OBJ3 / 2)
    bias[:, COL_LNH] = np.log(0.5)
    bias[:, BOFF_A:BOFF_A + 4] = _vec_to_pm(inputs["ba"], 4) * S_OBJ1
    bias[:, BOFF_M:BOFF_M + 4] = _vec_to_pm(inputs["bm"], 4) * S_CONDM
    bias[:, BOFF_Q:BOFF_Q + 4] = _vec_to_pm(inputs["bq"], 4)
    bias[:, BOFF_VM:BOFF_VM + 4] = _vec_to_pm(inputs["bvm"], 4) * S_VMP
    bias[:, BOFF_G:BOFF_G + 16] = _vec_to_pm(
        np.asarray(inputs["b_ih"], np.float32)
        + np.asarray(inputs["b_hh"], np.float32), 16)
    for si in range(len(SELS_M)):
        b = _vec_to_pm(inputs["b1"][si + 1], 4)
        bias[:, BOFF_1 + si * 4:BOFF_1 + si * 4 + 4] = b + np.log(S_OBJ2)
        bias[:, BOFF_1L + si * 4:BOFF_1L + si * 4 + 4] = b * S_OBJ2
    for si in range(len(SELS_Q)):
        b = _vec_to_pm(inputs["b2"][si + 1], 4)
        bias[:, BOFF_2 + si * 4:BOFF_2 + si * 4 + 4] = b + np.log(S_OBJ3 / 2)
        bias[:, BOFF_2L + si * 4:BOFF_2L + si * 4 + 4] = b * (S_OBJ3 / 2)
        bias[:, BOFF_G2 + si * 4:BOFF_G2 + si * 4 + 4] = _vec_to_pm(
            inputs["gb2"][si + 1], 4) / 2
    for si in range(len(SELS_VM)):
        b = _vec_to_pm(inputs["b3"][si + 1], 4)
        bias[:, BOFF_3 + si * 4:BOFF_3 + si * 4 + 4] = b
        bias[:, BOFF_3L + si * 4:BOFF_3L + si * 4 + 4] = b
    for si in range(len(SELS_VQ)):
        b = _vec_to_pm(inputs["b4"][si + 1], 4)
        bias[:, BOFF_4 + si * 4:BOFF_4 + si * 4 + 4] = b + np.log(0.5)
        bias[:, BOFF_4L + si * 4:BOFF_4L + si * 4 + 4] = b / 2
        bias[:, BOFF_G4 + si * 4:BOFF_G4 + si * 4 + 4] = _vec_to_pm(
            inputs["gb4"][si + 1], 4) / 2
    w["bias"] = bias
    return w


def _prep_core_inputs(inputs, core):
    b0 = core * BS
    app = np.asarray(inputs["appearance_video_feat"][b0:b0 + BS], np.float32)
    mot = np.asarray(inputs["motion_video_feat"][b0:b0 + BS], np.float32)
    q = np.asarray(inputs["question_embedding"][b0:b0 + BS], np.float32)
    # app [BS, C, F, V] -> [p, cc, half, pair, i, (f_h j)] e4m3 (x S_APP)
    app_t = app.transpose(3, 2, 0, 1).reshape(V, F, J)          # [V, F, J]
    app_t = app_t.reshape(8, 2, 128, F, J).transpose(2, 0, 1, 3, 4)  # [p,pr,i,F,J]
    app_t = app_t.reshape(128, 8, 2, 4, 2, 2, J)                # F -> cc,h,f_h
    app_t = app_t.transpose(0, 3, 4, 1, 2, 5, 6).reshape(128, 4, 2, 8, 2, 256)
    app_t = np.clip(app_t * S_APP, -240, 240)
    # mot [BS, C, V] -> [p, pair, i, j] e4m3 (x S_MOT)
    mot_t = mot.transpose(2, 0, 1).reshape(V, J)
    mot_t = mot_t.reshape(8, 2, 128, J).transpose(2, 0, 1, 3)
    mot_t = np.clip(mot_t * S_MOT, -240, 240)
    q_t = q.T.reshape(4, 128, BS).transpose(1, 0, 2)
    return {
        "app": np.ascontiguousarray(app_t).astype(E4),
        "mot": np.ascontiguousarray(mot_t).astype(E4),
        "q": np.ascontiguousarray(q_t).astype(BF),
    }


def _assemble(results):
    out = np.empty((B, (C - 4) * T, D), np.float32)
    for core in range(NCORES):
        r = np.asarray(results[core]["out"], np.float32).reshape(128, 4, 4, BS, T)
        # [p, si, m, b, t] -> [b, si, t, m, p]
        o = r.transpose(3, 1, 4, 2, 0).reshape(BS, (C - 4) * T, D)
        out[core * BS:(core + 1) * BS] = o
    return out


def build_in_maps(**inputs):
    w = _prep_weights(inputs)
    in_maps = []
    for core in range(NCORES):
        m = dict(w)
        m.update(_prep_core_inputs(inputs, core))
        in_maps.append(m)
    return in_maps


def _all_biases_zero(inputs):
    names = ["ba", "bm", "bq", "bvm", "b_ih", "b_hh", "b1", "b2", "gb2",
             "b3", "b4", "gb4"]
    return all(not np.any(np.asarray(inputs[n], np.float32)) for n in names)


def kernel(**inputs):
    nc = _program(_all_biases_zero(inputs))
    in_maps = build_in_maps(**inputs)
    res = run_bass_kernel_spmd(nc, in_maps, list(range(NCORES)))
    return _assemble(res.results)


if __name__ == "__main__":
    import reference

    inputs = {k: np.asarray(v) for k, v in reference.setup_inputs().items()}
    out = kernel(**inputs)
    exp = np.asarray(reference.reference(**inputs))
    err = np.abs(out - exp).max() / np.abs(exp).max()
    print("Relative error:", err)


# revision 13
# speedup vs baseline: 1.0136x; 1.0002x over previous
"""Trainium2 Bass kernel for nn_EncoderVidCRN (CRN video QA encoder), fp8 rev.

Data parallel over batch B=128 across 8 NeuronCores (16 rows/core). Mixed
precision chosen from a measured per-tensor error budget (rel gate 2e-2):

 - fp8-e4m3 (DoubleRow matmuls, 2 K-chunks/instruction): appearance proj
   (app, Wa), motion proj (mot, Wm), LSTM x-gates (W_ih), clip CRN banks
   W1/W2/gW2 and their moving operands (objsT, condm, condq, objs2T), video
   CRN-1 bank W3 with moving (clipT, vmc). Weight scale 2048, activation
   scales are fixed powers of two; inverse scales fold into epilogue imms.
 - fp8-e3m4 (plain matmuls, stationary only, scale 64): gW4, W_hh, Wvm.
 - bf16: q path (Wq) and the entire final CRN stage (W4, objs4T, qvc) --
   the only error-sensitive paths (measured: Wq or W4/mov4 in fp8 alone
   would each exceed half the error gate).

Subset means: rng subset choices replicated exactly (trace-time constants).
For the fp8 stages the subset sums run on the PE as extra accumulation
matmuls (direct sum of member objects, or full-sum S minus negated
complement when that needs fewer terms); 1/|sel| folds into the g-half of
each weight bank. The final stage sums on the vector engine in bf16.

ELU: s*elu(x) = max(s*x, min(s*e^x, s) - s) with s*e^x from one Exp
activation (bias ln s). Sigmoids (CRN gates and the LSTM) use the tanh
form sigmoid(x) = (tanh(x/2)+1)/2 so every activation (Exp/Tanh/Copy)
stays in the exp_and_others table set -- no act-table reloads. The LSTM
tracks c2=2c, h2=2h with W_hh/Wvm pre-halved so no extra ops are needed.
Biases are folded per-channel only when any bias input is nonzero (the
general path); the all-zero case (checked host-side) uses wide ops.
"""

import functools
import itertools
import sys

import numpy as np

sys.path.insert(0, "/opt/trn_rl_repo")

import ml_dtypes  # noqa: E402

import concourse.bass as bass  # noqa: E402,F401
import concourse.mybir as mybir  # noqa: E402
import concourse.tile as tile  # noqa: E402
from concourse import bacc  # noqa: E402
from concourse.bass_utils import run_bass_kernel_spmd  # noqa: E402

BF = ml_dtypes.bfloat16
E4 = ml_dtypes.float8_e4m3
E3 = ml_dtypes.float8_e3m4
B, C, F, V, D = 128, 8, 16, 2048, 512
NCORES = 8
BS = B // NCORES      # 16 batch rows per core
J = BS * C            # 128 clip-level columns per core
T = F - 4             # 12 retained time slots
JV = BS * T           # 192 video-level columns per core

F32 = mybir.dt.float32
BF16 = mybir.dt.bfloat16
FP8E4 = mybir.dt.float8e4
FP8E3 = mybir.dt.float8e3
AF = mybir.ActivationFunctionType
OP = mybir.AluOpType
DR = mybir.MatmulPerfMode.DoubleRow

# ---------------------------------------------------------------- scales
SW = 2048.0           # e4m3 weight scale
SW3 = 64.0            # e3m4 weight scale (gW4)
S_APP = 16.0
S_MOT = 16.0
S_OBJ1 = 4.0          # objsT / S1 family
S_CONDM = 8.0
S_QP = 32.0           # condq
S_OBJ2 = 8.0          # objs2T / S2 family
S_OBJ3 = 16.0         # clipT family
S_VMP = 128.0

IMM_OBJS = S_OBJ1 / (SW * S_APP)
IMM_CONDM = S_CONDM / (SW * S_MOT)
INV_GX = 1.0 / (SW * S_MOT)
INV_1 = 1.0 / (SW * S_OBJ1)
IMM_1 = S_OBJ2 * INV_1
INV_2 = 1.0 / (SW * S_OBJ2)
IMM_2 = S_OBJ3 * INV_2
INV_3 = 1.0 / (SW * S_OBJ3)
INV_G4 = 1.0 / SW3
INV_HH = 1.0 / SW3    # whh/wvm ship as e3m4 x 64

# ---------------------------------------------------------------- subsets


def _subsets():
    """Replicate the reference's rng sequence exactly (trace-time constant)."""
    rng = np.random.RandomState(0)
    out = []
    for n in (F, F - 2, C, C - 2):
        sels = []
        for scale_id in range(1, n - 1):
            scale = n - scale_id
            rels = list(itertools.combinations(range(n), scale))
            idx = rng.choice(len(rels), min(1, len(rels)), replace=False)
            sels.append(list(rels[int(idx[0])]))
        out.append(sels)
    return out


SELS_M, SELS_Q, SELS_VM, SELS_VQ = _subsets()

# bias table layout (f32 [128, NBIAS]); constants first, general-path
# per-channel bias columns after.
COL_LN2 = 0     # ln(S_OBJ2)
COL_LN3 = 1     # ln(S_OBJ3 / 2)  (gated: t_z carries s/2)
COL_LNH = 2     # ln(1/2)
BOFF_A, BOFF_M, BOFF_Q, BOFF_VM, BOFF_G = 4, 8, 12, 16, 20
BOFF_1 = 36             # 14*4  (b1 + ln S_OBJ2 for exp; raw*s2 in BOFF_1L)
BOFF_1L = 92            # 14*4  (b1 * S_OBJ2)
BOFF_2 = 148            # 12*4  (b2 + ln(S_OBJ3/2))
BOFF_2L = 196           # 12*4  (b2 * S_OBJ3/2)
BOFF_G2 = 244           # 12*4  (gb2/2, tanh-form gate bias)
BOFF_3 = 292            # 6*4   (b3; exp bias, ln1=0)
BOFF_3L = 316           # 6*4   (b3)
BOFF_4 = 340            # 4*4   (b4 + ln(1/2))
BOFF_4L = 356           # 4*4   (b4/2)
BOFF_G4 = 372           # 4*4   (gb4/2)
NBIAS = 388


def _use_comp(n, sel):
    return (n - len(sel)) + 1 < len(sel)

# ---------------------------------------------------------------- device IR


def _gunits(n, sel, slicer, s_ap, neg_slicer):
    """Moving-operand list for the PE-side subset sum of `sel` over n objects:
    either the member slices, or [S] + negated complement slices."""
    if _use_comp(n, sel):
        in_set = set(sel)
        return [s_ap] + [neg_slicer(i) for i in range(n) if i not in in_set]
    return [slicer(f) for f in sel]


def _dr_group(nc, ps_m, wt_g, wt_c, units, cond_pairs, mslice, cond_first=True):
    """One PSUM accumulation group of DoubleRow matmuls: the cond pairs plus
    the g units. cond_first puts the cond matmuls first (their operands are
    usually ready early, keeping the PE busy across phase barriers); crn_vm
    uses cond_first=False because vmc lands late (after the LSTM chain).
    wt_g/wt_c: [128, 2, 2, 512] stationary halves; units/cond_pairs: lists of
    per-pair moving APs ([128, 2, N])."""
    total = len(units) * 2 + 2
    ops = []
    for u in units:
        for p in range(2):
            ops.append((wt_g[:, p, :, mslice], u[p]))
    cond_ops = [(wt_c[:, p, :, mslice], cond_pairs[p]) for p in range(2)]
    ops = cond_ops + ops if cond_first else ops + cond_ops
    for k, (w, x) in enumerate(ops):
        nc.tensor.matmul(ps_m, w, x, start=(k == 0), stop=(k == total - 1),
                         perf_mode=DR)


def _pairs(ap4):
    """[128, 2, 2, N] AP -> per-pair [128, 2, N] moving APs."""
    return [ap4[:, 0, :, :], ap4[:, 1, :, :]]


@functools.lru_cache(maxsize=2)
def _program(biasfree=True, debug=False):
    nc = bacc.Bacc("TRN2", target_bir_lowering=False, debug=False,
                   num_devices=NCORES)

    app_d = nc.dram_tensor("app", [128, 4, 2, 8, 2, 256], FP8E4, kind="ExternalInput")
    mot_d = nc.dram_tensor("mot", [128, 8, 2, J], FP8E4, kind="ExternalInput")
    q_d = nc.dram_tensor("q", [128, 4, BS], BF16, kind="ExternalInput")
    wa_d = nc.dram_tensor("wa", [128, 8, 2, 512], FP8E4, kind="ExternalInput")
    wm_d = nc.dram_tensor("wm", [128, 8, 2, 512], FP8E4, kind="ExternalInput")
    wq_d = nc.dram_tensor("wq", [128, 4, 512], BF16, kind="ExternalInput")
    wvm_d = nc.dram_tensor("wvm", [128, 4, 512], FP8E3, kind="ExternalInput")
    wih_d = nc.dram_tensor("wih", [128, 16, 8, 2, 128], FP8E4, kind="ExternalInput")
    whh_d = nc.dram_tensor("whh", [128, 4, 2048], FP8E3, kind="ExternalInput")
    w1_d = nc.dram_tensor("w1", [128, 14, 2, 2, 2, 512], FP8E4, kind="ExternalInput")
    w2_d = nc.dram_tensor("w2", [128, 12, 2, 2, 2, 512], FP8E4, kind="ExternalInput")
    w2g_d = nc.dram_tensor("w2g", [128, 12, 2, 2, 2, 512], FP8E4, kind="ExternalInput")
    w3_d = nc.dram_tensor("w3", [128, 6, 2, 2, 2, 512], FP8E4, kind="ExternalInput")
    w4_d = nc.dram_tensor("w4", [128, 4, 8, 512], BF16, kind="ExternalInput")
    w4g_d = nc.dram_tensor("w4g", [128, 4, 8, 512], FP8E3, kind="ExternalInput")
    bias_d = nc.dram_tensor("bias", [128, NBIAS], F32, kind="ExternalInput")
    out_d = nc.dram_tensor("out", [128, 4, 4 * JV], BF16, kind="ExternalOutput")

    nc._phases = []

    def _mark(name):
        nc._phases.append((name, int(nc.get_next_instruction_name()[2:])))

    with tile.TileContext(nc) as tc:
        # Pools form a strict stack (release order = reverse of allocation).
        perm = tc.alloc_tile_pool(name="perm", bufs=1)
        tpool = tc.alloc_tile_pool(name="tmp", bufs=4)
        stream = tc.alloc_tile_pool(name="stream", bufs=4)
        p5 = tc.alloc_tile_pool(name="p5", bufs=1)        # clipT
        p4 = tc.alloc_tile_pool(name="p4", bufs=1)        # objs2T (+neg, S2)
        p3 = tc.alloc_tile_pool(name="p3", bufs=1)        # objsT (+neg, S1), condm
        p0 = tc.alloc_tile_pool(name="p0", bufs=1)        # early consts
        pp_early = tc.alloc_tile_pool(name="ps_early", bufs=1, space="PSUM")

        _mark("consts")
        bias = perm.tile([128, NBIAS], F32, name="bias")
        nc.sync.dma_start(bias, bias_d[:])

        def bap(off):
            return bias[:, off:off + 1]

        motT = p0.tile([128, 8, 2, J], FP8E4, name="motT")
        nc.sync.dma_start(motT, mot_d[:])
        qT = p0.tile([128, 4, BS], BF16, name="qT")
        nc.sync.dma_start(qT, q_d[:])
        wqt = p0.tile([128, 4, 512], BF16, name="wqt")
        nc.sync.dma_start(wqt, wq_d[:])

        _mark("qproj_condm")
        # ---------------- q_proj (bf16) -> qp [128, 4, BS]
        psq = pp_early.tile([128, 4, BS], F32, tag="psq", name="psq")
        for m in range(4):
            for kc in range(4):
                nc.tensor.matmul(psq[:, m, :], wqt[:, kc, m * 128:(m + 1) * 128],
                                 qT[:, kc, :], start=(kc == 0), stop=(kc == 3))
        qp = perm.tile([128, 4, BS], BF16, name="qp")
        if biasfree:
            nc.vector.tensor_copy(qp, psq)
        else:
            for m in range(4):
                nc.vector.tensor_scalar_add(qp[:, m, :], psq[:, m, :],
                                            bap(BOFF_Q + m))

        # ---------------- mot_proj (DR) -> condm [128, 2, 2, J] e4m3
        wmt = p0.tile([128, 8, 2, 512], FP8E4, name="wmt")
        nc.sync.dma_start(wmt, wm_d[:])
        pscm = pp_early.tile([128, 4, J], F32, tag="pscm", name="pscm")
        for m in range(4):
            for p in range(8):
                nc.tensor.matmul(pscm[:, m, :], wmt[:, p, :, m * 128:(m + 1) * 128],
                                 motT[:, p, :, :], start=(p == 0), stop=(p == 7),
                                 perf_mode=DR)
        condm = p3.tile([128, 2, 2, J], FP8E4, name="condm")
        condm_w = condm.rearrange("p a b j -> p (a b) j")
        if biasfree:
            nc.vector.tensor_scalar_mul(condm_w, pscm, IMM_CONDM)
        else:
            for m in range(4):
                nc.vector.tensor_scalar(condm_w[:, m, :], pscm[:, m, :],
                                        IMM_CONDM, bap(BOFF_M + m),
                                        OP.mult, OP.add)

        # cond broadcasts: condq e4m3 (x S_QP), qvc bf16
        condq = perm.tile([128, 2, 2, BS, C], FP8E4, name="condq")
        nc.vector.tensor_scalar_mul(
            condq.rearrange("p a b s c -> p (a b) s c"),
            qp[:, :, :, None].to_broadcast([128, 4, BS, C]), S_QP)
        qvc = perm.tile([128, 4, BS, T], BF16, name="qvc")
        nc.vector.tensor_copy(qvc, qp[:, :, :, None].to_broadcast([128, 4, BS, T]))
        qvc_v = qvc.rearrange("p d b t -> p d (b t)")
        pp_early.release()

        _mark("stageA")
        # ---------------- stage A: app_proj (DR) -> objsT/neg [128,2,2,F,J]
        p2 = tc.alloc_tile_pool(name="p2", bufs=1)
        apps = tc.alloc_tile_pool(name="apps", bufs=3)
        pp_a = tc.alloc_tile_pool(name="ps_a", bufs=2, space="PSUM")
        wat = p2.tile([128, 8, 2, 512], FP8E4, name="wat")
        nc.sync.dma_start(wat, wa_d[:])
        objsT = p3.tile([128, 2, 2, F, J], FP8E4, name="objsT")
        nobjsT = p3.tile([128, 2, 2, F, J], FP8E4, name="nobjsT")
        for cc in range(4):
            xc = apps.tile([128, 2, 8, 2, 256], FP8E4, tag="app", name="xc")
            nc.sync.dma_start(xc, app_d[:, cc, :, :, :, :])
            for m in range(4):
                ps_a = pp_a.tile([128, 512], F32, tag="psA", name="ps_a")
                for h in range(2):
                    for p in range(8):
                        nc.tensor.matmul(ps_a[:, h * 256:(h + 1) * 256],
                                         wat[:, p, :, m * 128:(m + 1) * 128],
                                         xc[:, h, p, :, :],
                                         start=(p == 0), stop=(p == 7),
                                         perf_mode=DR)
                dst = objsT[:, m // 2, m % 2, cc * 4:(cc + 1) * 4, :]
                dst = dst.rearrange("p f j -> p (f j)")
                ndst = nobjsT[:, m // 2, m % 2, cc * 4:(cc + 1) * 4, :]
                ndst = ndst.rearrange("p f j -> p (f j)")
                if biasfree:
                    nc.vector.tensor_scalar_mul(dst, ps_a, IMM_OBJS)
                else:
                    nc.vector.tensor_scalar(dst, ps_a, IMM_OBJS,
                                            bap(BOFF_A + m), OP.mult, OP.add)
                nc.scalar.mul(ndst, dst, -1.0)
        pp_a.release()
        apps.release()
        p2.release()

        # S1 = sum_f objsT (two-accumulator bf16 chain, final e4m3)
        s1 = p3.tile([128, 2, 2, J], FP8E4, name="s1")
        s1a = p3.tile([128, 2, 2, J], BF16, name="s1a")
        s1b = p3.tile([128, 2, 2, J], BF16, name="s1b")
        nc.vector.tensor_add(s1a, objsT[:, :, :, 0, :], objsT[:, :, :, 1, :])
        nc.vector.tensor_add(s1b, objsT[:, :, :, 2, :], objsT[:, :, :, 3, :])
        for f in range(4, F):
            t = s1a if f % 2 == 0 else s1b
            nc.vector.tensor_add(t, t, objsT[:, :, :, f, :])
        nc.vector.tensor_add(s1, s1a, s1b)

        _mark("crn_m")
        # ---------------- crn_m: objsT -> objs2T [128, 2, 2, 14, J]
        pp_crn = tc.alloc_tile_pool(name="ps_crn", bufs=2, space="PSUM")
        objs2T = p4.tile([128, 2, 2, 14, J], FP8E4, name="objs2T")
        nobjs2T = p4.tile([128, 2, 2, 14, J], FP8E4, name="nobjs2T")
        s2a = p4.tile([128, 2, 2, J], BF16, name="s2a")

        def obj1(f):
            return _pairs(objsT[:, :, :, f, :])

        def nobj1(f):
            return _pairs(nobjsT[:, :, :, f, :])

        # complement scales last so S1/neg have time to complete
        order_m = ([i for i, s in enumerate(SELS_M) if not _use_comp(F, s)]
                   + [i for i, s in enumerate(SELS_M) if _use_comp(F, s)])
        cond_m_pairs = _pairs(condm)
        for oi, si in enumerate(order_m):
            sel = SELS_M[si]
            w1t = stream.tile([128, 2, 2, 2, 512], FP8E4, tag="crnw", name="w1t", bufs=10)
            nc.sync.dma_start(w1t, w1_d[:, si, :, :, :, :])
            if _use_comp(F, sel):
                in_set = set(sel)
                units = [_pairs(s1)] + [nobj1(i) for i in range(F)
                                        if i not in in_set]
            else:
                units = [obj1(f) for f in sel]
            ps = pp_crn.tile([128, 4, J], F32, tag="psM", name="ps_m1", bufs=3)
            for m in range(4):
                _dr_group(nc, ps[:, m, :], w1t[:, 0], w1t[:, 1], units,
                          cond_m_pairs, slice(m * 128, (m + 1) * 128))
            # epilogue: objs2T[si] = S_OBJ2 * elu(inv1 * ps + b)
            t_e = tpool.tile([128, 4, J], BF16, tag="t_exp", name="t_e", bufs=3)
            t_m = tpool.tile([128, 4, J], BF16, tag="t_min", name="t_m", bufs=3)
            dst = objs2T[:, :, :, si, :].rearrange("p a b j -> p (a b) j")
            ndst = nobjs2T[:, :, :, si, :].rearrange("p a b j -> p (a b) j")
            if biasfree:
                nc.scalar.activation(t_e, ps, AF.Exp, bias=bap(COL_LN2),
                                     scale=INV_1)
                nc.vector.tensor_scalar(t_m, t_e, S_OBJ2, -S_OBJ2, OP.min, OP.add)
                nc.vector.scalar_tensor_tensor(dst, ps, IMM_1, t_m,
                                               OP.mult, OP.max)
            else:
                for m in range(4):
                    nc.scalar.activation(t_e[:, m, :], ps[:, m, :], AF.Exp,
                                         bias=bap(BOFF_1 + si * 4 + m),
                                         scale=INV_1)
                nc.vector.tensor_scalar(t_m, t_e, S_OBJ2, -S_OBJ2, OP.min, OP.add)
                for m in range(4):
                    lin = tpool.tile([128, J], F32, tag="lin", name="lin")
                    nc.vector.tensor_scalar(lin, ps[:, m, :], IMM_1,
                                            bap(BOFF_1L + si * 4 + m),
                                            OP.mult, OP.add)
                    nc.vector.tensor_tensor(dst[:, m, :], lin, t_m[:, m, :], OP.max)
            nc.scalar.mul(ndst, dst, -1.0)
            # incremental S2
            s2src = objs2T[:, :, :, si, :]
            if oi == 0:
                nc.vector.tensor_copy(s2a, s2src)
            else:
                nc.vector.tensor_add(s2a, s2a, s2src)
        s2 = p4.tile([128, 2, 2, J], FP8E4, name="s2")
        nc.vector.tensor_copy(s2, s2a)

        _mark("gatesx")
        # ---------------- LSTM x-gates: gx = inv * (W_ih @ motT)  (DR)
        wihs = tc.alloc_tile_pool(name="wihs", bufs=10)
        p1 = tc.alloc_tile_pool(name="p1", bufs=1)
        ppx = tc.alloc_tile_pool(name="ps_x", bufs=2, space="PSUM")
        whht = p1.tile([128, 4, 2048], FP8E3, name="whht")
        nc.sync.dma_start(whht, whh_d[:])
        wvmt = p1.tile([128, 4, 512], FP8E3, name="wvmt")
        nc.sync.dma_start(wvmt, wvm_d[:])
        gx = p1.tile([128, 16, J], F32, name="gx")
        for mi in range(16):
            wih_t = wihs.tile([128, 8, 2, 128], FP8E4, tag="wih", name="wih_t")
            nc.sync.dma_start(wih_t, wih_d[:, mi, :, :, :])
            psx = ppx.tile([128, J], F32, tag="psx", name="psx")
            for p in range(8):
                nc.tensor.matmul(psx, wih_t[:, p, :, :], motT[:, p, :, :],
                                 start=(p == 0), stop=(p == 7), perf_mode=DR)
            if biasfree:
                nc.vector.tensor_scalar_mul(gx[:, mi, :], psx, INV_GX)
            else:
                nc.vector.tensor_scalar(gx[:, mi, :], psx, INV_GX,
                                        bap(BOFF_G + mi), OP.mult, OP.add)
        ppx.release()
        pp_r = tc.alloc_tile_pool(name="ps_r", bufs=2, space="PSUM")
        gxr = gx.rearrange("p m (b c) -> p m c b", c=C)

        _mark("lstm")
        # ---------------- LSTM recurrence, tanh-only form (one act table):
        # sigmoid(x) = (tanh(x/2)+1)/2. Track c2 = 2c and h2 = 2h; the /2 of
        # each sigmoid folds into stt imms and W_hh/Wvm are pre-halved on the
        # host so psr = W_hh @ h exactly.
        h_prev = None
        c_prev = None
        for t in range(C):
            xg = gxr[:, :, t, :]
            if t == 0:
                gates = xg
            else:
                psr = pp_r.tile([128, 16, BS], F32, tag="psr", name="psr")
                for mi in range(16):
                    for kc in range(4):
                        nc.tensor.matmul(psr[:, mi, :],
                                         whht[:, kc, mi * 128:(mi + 1) * 128],
                                         h_prev[:, kc, :],
                                         start=(kc == 0), stop=(kc == 3))
                gates = tpool.tile([128, 16, BS], F32, tag="lstm_g", name="lstm_g")
                nc.vector.scalar_tensor_tensor(gates, psr, INV_HH, xg,
                                               OP.mult, OP.add)
            t_if = tpool.tile([128, 8, BS], F32, tag="dif", name="t_if")
            nc.scalar.activation(t_if, gates[:, 0:8, :], AF.Tanh, scale=0.5)
            tan_g = tpool.tile([128, 4, BS], F32, tag="tg", name="tan_g")
            nc.scalar.activation(tan_g, gates[:, 8:12, :], AF.Tanh)
            t_o = tpool.tile([128, 4, BS], F32, tag="do", name="t_o")
            nc.scalar.activation(t_o, gates[:, 12:16, :], AF.Tanh, scale=0.5)
            # ig2 = (tanh_i+1)*tan_g = 2*sig_i*tan_g
            ig2 = tpool.tile([128, 4, BS], F32, tag="ig", name="ig2", bufs=2)
            nc.vector.scalar_tensor_tensor(ig2, t_if[:, 0:4, :], 1.0, tan_g,
                                           OP.add, OP.mult)
            if t == 0:
                c2_t = ig2
            else:
                # fc2 = (tanh_f+1)*c2_prev = 4*sig_f*c ; c2 = fc2/2 + ig2
                c2_t = tpool.tile([128, 4, BS], F32, tag="c_t", name="c2_t", bufs=2)
                fc2 = tpool.tile([128, 4, BS], F32, tag="fc", name="fc2")
                nc.vector.scalar_tensor_tensor(fc2, t_if[:, 4:8, :], 1.0,
                                               c_prev, OP.add, OP.mult)
                nc.vector.scalar_tensor_tensor(c2_t, fc2, 0.5, ig2,
                                               OP.mult, OP.add)
            tan_c = tpool.tile([128, 4, BS], F32, tag="tanc", name="tan_c")
            nc.scalar.activation(tan_c, c2_t, AF.Tanh, scale=0.5)
            # h2 = (tanh_o+1)*tanh(c) = 2h
            h_t = tpool.tile([128, 4, BS], BF16, tag="h_t", name="h2_t", bufs=2)
            nc.vector.scalar_tensor_tensor(h_t, t_o, 1.0, tan_c,
                                           OP.add, OP.mult)
            h_prev, c_prev = h_t, c2_t

        # vm_proj (bf16) -> vmc [128, 2, 2, BS, T] e4m3 (x S_VMP)
        psv = pp_r.tile([128, 4, BS], F32, tag="psv", name="psv", bufs=1)
        for m in range(4):
            for kc in range(4):
                nc.tensor.matmul(psv[:, m, :], wvmt[:, kc, m * 128:(m + 1) * 128],
                                 h_prev[:, kc, :], start=(kc == 0), stop=(kc == 3))
        vmp = p1.tile([128, 4, BS], FP8E4, name="vmp")
        if biasfree:
            nc.vector.tensor_scalar_mul(vmp, psv, S_VMP * INV_HH)
        else:
            for m in range(4):
                nc.vector.tensor_scalar(vmp[:, m, :], psv[:, m, :], S_VMP * INV_HH,
                                        bap(BOFF_VM + m), OP.mult, OP.add)
        vmc = perm.tile([128, 2, 2, BS, T], FP8E4, name="vmc")
        nc.vector.tensor_copy(
            vmc.rearrange("p a b s t -> p (a b) s t"),
            vmp[:, :, :, None].to_broadcast([128, 4, BS, T]))
        pp_r.release()
        p1.release()
        wihs.release()

        _mark("crn_q")
        # ---------------- crn_q (gated): objs2T -> clipT [128,2,2,C,BS,T]
        clipT = p5.tile([128, 2, 2, C, BS, T], FP8E4, name="clipT")

        def obj2(s):
            return _pairs(objs2T[:, :, :, s, :])

        def nobj2(s):
            return _pairs(nobjs2T[:, :, :, s, :])

        order_q = ([i for i, s in enumerate(SELS_Q) if not _use_comp(F - 2, s)]
                   + [i for i, s in enumerate(SELS_Q) if _use_comp(F - 2, s)])
        condq_pairs = _pairs(condq.rearrange("p a b s c -> p a b (s c)"))
        for si in order_q:
            sel = SELS_Q[si]
            w2t = stream.tile([128, 2, 2, 2, 512], FP8E4, tag="crnw", name="w2t", bufs=10)
            nc.sync.dma_start(w2t, w2_d[:, si, :, :, :, :])
            w2gt = stream.tile([128, 2, 2, 2, 512], FP8E4, tag="crnw", name="w2gt", bufs=10)
            nc.sync.dma_start(w2gt, w2g_d[:, si, :, :, :, :])
            if _use_comp(F - 2, sel):
                in_set = set(sel)
                units = [_pairs(s2)] + [nobj2(i) for i in range(F - 2)
                                        if i not in in_set]
            else:
                units = [obj2(s) for s in sel]
            ps_m = pp_crn.tile([128, 4, J], F32, tag="psM", name="ps_q1", bufs=3)
            ps_g = pp_crn.tile([128, 4, J], F32, tag="psG", name="ps_q2")
            for m in range(4):
                _dr_group(nc, ps_m[:, m, :], w2t[:, 0], w2t[:, 1], units,
                          condq_pairs, slice(m * 128, (m + 1) * 128))
            for m in range(4):
                _dr_group(nc, ps_g[:, m, :], w2gt[:, 0], w2gt[:, 1], units,
                          condq_pairs, slice(m * 128, (m + 1) * 128))
            # gated epilogue, tanh form: t_z carries S_OBJ3/2 * elu;
            # out = (tanh(gate/2)+1) * t_z = S_OBJ3 * elu * sigmoid(gate)
            t_e = tpool.tile([128, 4, J], BF16, tag="t_exp", name="t_e", bufs=3)
            t_m = tpool.tile([128, 4, J], BF16, tag="t_min", name="t_m", bufs=3)
            t_z = tpool.tile([128, 4, J], BF16, tag="t_z", name="t_z", bufs=3)
            t_d = tpool.tile([128, 4, J], BF16, tag="t_d", name="t_d", bufs=3)
            h3 = S_OBJ3 / 2
            if biasfree:
                nc.scalar.activation(t_e, ps_m, AF.Exp, bias=bap(COL_LN3),
                                     scale=INV_2)
                nc.vector.tensor_scalar(t_m, t_e, h3, -h3, OP.min, OP.add)
                nc.vector.scalar_tensor_tensor(t_z, ps_m, IMM_2 / 2, t_m,
                                               OP.mult, OP.max)
                nc.scalar.activation(t_d, ps_g, AF.Tanh, scale=INV_2 / 2)
            else:
                for m in range(4):
                    nc.scalar.activation(t_e[:, m, :], ps_m[:, m, :], AF.Exp,
                                         bias=bap(BOFF_2 + si * 4 + m),
                                         scale=INV_2)
                    nc.scalar.activation(t_d[:, m, :], ps_g[:, m, :], AF.Tanh,
                                         bias=bap(BOFF_G2 + si * 4 + m),
                                         scale=INV_2 / 2)
                nc.vector.tensor_scalar(t_m, t_e, h3, -h3, OP.min, OP.add)
                for m in range(4):
                    lin = tpool.tile([128, J], F32, tag="lin", name="lin")
                    nc.vector.tensor_scalar(lin, ps_m[:, m, :], IMM_2 / 2,
                                            bap(BOFF_2L + si * 4 + m),
                                            OP.mult, OP.add)
                    nc.vector.tensor_tensor(t_z[:, m, :], lin, t_m[:, m, :], OP.max)
            # dst view: cols j=(b c) -> clipT[:, :, :, c, b, si].
            # (t_d + 1) via a 3D-out tensor_scalar first: ScalarTensorTensor
            # outputs must be <= 3D and the clipT view is 4D.
            nc.vector.tensor_scalar_add(t_d, t_d, 1.0)
            wide = clipT[:, :, :, :, :, si].rearrange("p a b c s -> p (a b) s c")
            nc.vector.tensor_tensor(wide, t_d.rearrange("p d (s c) -> p d s c", c=C),
                                    t_z.rearrange("p d (s c) -> p d s c", c=C),
                                    OP.mult)
        pp_crn.release()
        p0.release()
        p3.release()
        p4.release()

        _mark("crn_vm")
        # ---------------- crn_vm (ungated, direct sums): clipT -> objs4T bf16
        pp_v = tc.alloc_tile_pool(name="ps_v", bufs=1, space="PSUM")
        objs4T = perm.tile([128, 4, 6, JV], BF16, name="objs4T")

        def clip_pairs(c):
            ap = clipT[:, :, :, c, :, :]
            return _pairs(ap.rearrange("p a b s t -> p a b (s t)"))

        vmc_pairs = _pairs(vmc.rearrange("p a b s t -> p a b (s t)"))
        # incremental S4 accumulator (bf16) so crn_vq's complement scales can
        # start right after the last crn_vm epilogue
        s4 = perm.tile([128, 4, JV], BF16, name="s4")
        for si, sel in enumerate(SELS_VM):
            w3t = stream.tile([128, 2, 2, 2, 512], FP8E4, tag="crnw", name="w3t", bufs=10)
            nc.sync.dma_start(w3t, w3_d[:, si, :, :, :, :])
            units = [clip_pairs(c) for c in sel]
            # alternate tag pairs so 4 scale-epilogues can be in flight
            # (psV2/3 are otherwise idle until crn_vq)
            t0, t1 = ("psV0", "psV1") if si % 2 == 0 else ("psV2", "psV3")
            ps0 = pp_v.tile([128, 2, JV], F32, tag=t0, name="ps_vm0", bufs=2)
            ps1 = pp_v.tile([128, 2, JV], F32, tag=t1, name="ps_vm1", bufs=2)
            ps_list = [ps0[:, 0, :], ps0[:, 1, :], ps1[:, 0, :], ps1[:, 1, :]]
            for m in range(4):
                _dr_group(nc, ps_list[m], w3t[:, 0], w3t[:, 1], units,
                          vmc_pairs, slice(m * 128, (m + 1) * 128))
            for half, psh in enumerate((ps0, ps1)):
                t_e = tpool.tile([128, 2, JV], BF16, tag="t_expv", name="t_ev", bufs=3)
                t_m = tpool.tile([128, 2, JV], BF16, tag="t_minv", name="t_mv", bufs=3)
                dst = objs4T[:, half * 2:(half + 1) * 2, si, :]
                if biasfree:
                    nc.scalar.activation(t_e, psh, AF.Exp, scale=INV_3)
                    nc.vector.tensor_scalar(t_m, t_e, 1.0, -1.0, OP.min, OP.add)
                    nc.vector.scalar_tensor_tensor(dst, psh, INV_3, t_m,
                                                   OP.mult, OP.max)
                else:
                    for mm in range(2):
                        m = half * 2 + mm
                        nc.scalar.activation(t_e[:, mm, :], psh[:, mm, :], AF.Exp,
                                             bias=bap(BOFF_3 + si * 4 + m),
                                             scale=INV_3)
                        nc.vector.tensor_scalar(t_m[:, mm, :], t_e[:, mm, :],
                                                1.0, -1.0, OP.min, OP.add)
                        lin = tpool.tile([128, JV], F32, tag="linv", name="linv")
                        nc.vector.tensor_scalar(lin, psh[:, mm, :], INV_3,
                                                bap(BOFF_3L + si * 4 + m),
                                                OP.mult, OP.add)
                        nc.vector.tensor_tensor(dst[:, mm, :], lin, t_m[:, mm, :],
                                                OP.max)
            # incremental S4
            s4src = objs4T[:, :, si, :]
            if si == 0:
                nc.vector.tensor_copy(s4, s4src)
            else:
                nc.vector.tensor_add(s4, s4, s4src)

        _mark("crn_vq")
        # ---------------- crn_vq (bf16, gated): objs4T -> out
        def o4_slice(s):
            return objs4T[:, :, s, :]

        gpool = tc.alloc_tile_pool(name="gpool", bufs=4)
        # direct-sum scales first: they don't need s4
        order_vq = ([i for i, s in enumerate(SELS_VQ)
                     if not (C - 2 - len(s)) + 1 < len(s)]
                    + [i for i, s in enumerate(SELS_VQ)
                       if (C - 2 - len(s)) + 1 < len(s)])
        for si in order_vq:
            sel = SELS_VQ[si]
            w4t = stream.tile([128, 8, 512], BF16, tag="crnw4", name="w4t", bufs=2)
            nc.sync.dma_start(w4t, w4_d[:, si, :, :])
            w4gt = stream.tile([128, 8, 512], FP8E3, tag="crnw4g", name="w4gt", bufs=2)
            nc.sync.dma_start(w4gt, w4g_d[:, si, :, :])
            # g = subset sum (bf16 DVE, complement vs direct)
            in_set = set(sel)
            comp = [i for i in range(C - 2) if i not in in_set]
            if len(comp) + 1 < len(sel):
                g = gpool.tile([128, 4, JV], BF16, tag="g4", name="g4")
                nc.vector.tensor_sub(g, s4, o4_slice(comp[0]))
                for i in comp[1:]:
                    nc.vector.tensor_sub(g, g, o4_slice(i))
            elif len(sel) == 1:
                g = o4_slice(sel[0])
            else:
                g = gpool.tile([128, 4, JV], BF16, tag="g4", name="g4")
                nc.vector.tensor_add(g, o4_slice(sel[0]), o4_slice(sel[1]))
                for i in sel[2:]:
                    nc.vector.tensor_add(g, g, o4_slice(i))
            ps0 = pp_v.tile([128, 2, JV], F32, tag="psV0", name="ps_vq0", bufs=2)
            ps1 = pp_v.tile([128, 2, JV], F32, tag="psV1", name="ps_vq1", bufs=2)
            pg0 = pp_v.tile([128, 2, JV], F32, tag="psV2", name="ps_vq2", bufs=2)
            pg1 = pp_v.tile([128, 2, JV], F32, tag="psV3", name="ps_vq3", bufs=2)
            ps_list = [ps0[:, 0, :], ps0[:, 1, :], ps1[:, 0, :], ps1[:, 1, :]]
            pg_list = [pg0[:, 0, :], pg0[:, 1, :], pg1[:, 0, :], pg1[:, 1, :]]
            for m in range(4):
                msl = slice(m * 128, (m + 1) * 128)
                for kc in range(4):
                    nc.tensor.matmul(ps_list[m], w4t[:, 4 + kc, msl],
                                     qvc_v[:, kc, :], start=(kc == 0), stop=False)
                for kc in range(4):
                    nc.tensor.matmul(ps_list[m], w4t[:, kc, msl], g[:, kc, :],
                                     start=False, stop=(kc == 3))
            for m in range(4):
                msl = slice(m * 128, (m + 1) * 128)
                for kc in range(4):
                    nc.tensor.matmul(pg_list[m], w4gt[:, 4 + kc, msl],
                                     qvc_v[:, kc, :], start=(kc == 0), stop=False)
                for kc in range(4):
                    nc.tensor.matmul(pg_list[m], w4gt[:, kc, msl], g[:, kc, :],
                                     start=False, stop=(kc == 3))
            ot = tpool.tile([128, 4, JV], BF16, tag="ot", name="ot4", bufs=2)
            for half, (psh, pgh) in enumerate(((ps0, pg0), (ps1, pg1))):
                t_e = tpool.tile([128, 2, JV], BF16, tag="t_expv", name="t_ev", bufs=3)
                t_m = tpool.tile([128, 2, JV], BF16, tag="t_minv", name="t_mv", bufs=3)
                t_z = tpool.tile([128, 2, JV], BF16, tag="t_zv", name="t_zv", bufs=3)
                t_d = tpool.tile([128, 2, JV], BF16, tag="t_dv", name="t_dv", bufs=3)
                oth = ot[:, half * 2:(half + 1) * 2, :]
                if biasfree:
                    nc.scalar.activation(t_e, psh, AF.Exp, bias=bap(COL_LNH))
                    nc.vector.tensor_scalar(t_m, t_e, 0.5, -0.5, OP.min, OP.add)
                    nc.vector.scalar_tensor_tensor(t_z, psh, 0.5, t_m,
                                                   OP.mult, OP.max)
                    nc.scalar.activation(t_d, pgh, AF.Tanh, scale=INV_G4 / 2)
                    nc.vector.scalar_tensor_tensor(oth, t_d, 1.0, t_z,
                                                   OP.add, OP.mult)
                else:
                    for mm in range(2):
                        m = half * 2 + mm
                        nc.scalar.activation(t_e[:, mm, :], psh[:, mm, :], AF.Exp,
                                             bias=bap(BOFF_4 + si * 4 + m))
                        nc.scalar.activation(t_d[:, mm, :], pgh[:, mm, :],
                                             AF.Tanh,
                                             bias=bap(BOFF_G4 + si * 4 + m),
                                             scale=INV_G4 / 2)
                        nc.vector.tensor_scalar(t_m[:, mm, :], t_e[:, mm, :],
                                                0.5, -0.5, OP.min, OP.add)
                        lin = tpool.tile([128, JV], F32, tag="linv", name="linv")
                        nc.vector.tensor_scalar(lin, psh[:, mm, :], 0.5,
                                                bap(BOFF_4L + si * 4 + m),
                                                OP.mult, OP.add)
                        nc.vector.tensor_tensor(t_z[:, mm, :], lin, t_m[:, mm, :],
                                                OP.max)
                        nc.vector.scalar_tensor_tensor(oth[:, mm, :],
                                                       t_d[:, mm, :], 1.0,
                                                       t_z[:, mm, :],
                                                       OP.add, OP.mult)
            nc.sync.dma_start(out_d[:, si, :], ot.rearrange("p d j -> p (d j)"))

        for pool in (gpool, pp_v, p5, stream, tpool, perm):
            pool.release()

    nc.compile()
    return nc


# ---------------------------------------------------------------- host side


def _kxm_pairs(w_t, scale, dt):
    """[K, M] f32 -> [128, K//256, 2, M] scaled/clipped fp8 (pair-grouped)."""
    K, M = w_t.shape
    lim = 240.0 if dt is E4 else 15.5
    w = np.clip(np.asarray(w_t, np.float32) * scale, -lim, lim)
    return np.ascontiguousarray(
        w.reshape(K // 256, 2, 128, M).transpose(2, 0, 1, 3)).astype(dt)


def _bank_dr(Ws, sels, s_cond_ratio):
    """CRN bank -> [128, S, 2, 2, 2, 512] e4m3: halves (g/|sel|, c*ratio)."""
    per = []
    for si, sel in enumerate(sels):
        s_id = si + 1
        hg = np.asarray(Ws[s_id][:, :D], np.float32).T / len(sel) * SW
        hc = np.asarray(Ws[s_id][:, D:], np.float32).T * (SW * s_cond_ratio)
        h = np.stack([hg, hc])  # [2, 512, 512]
        h = np.clip(h, -240, 240)
        per.append(h.reshape(2, 2, 2, 128, 512).transpose(3, 0, 1, 2, 4))
    return np.ascontiguousarray(np.stack(per, axis=1)).astype(E4)


def _bank_bf16(Ws, sels, dt=BF, scale=1.0):
    """Stage-4 bank -> [128, S, 8, 512] (halves g/|sel|, c as 4+4 k-chunks)."""
    lim = {BF: 3e38, E3: 15.5, E4: 240.0}[dt]
    per = []
    for si, sel in enumerate(sels):
        s_id = si + 1
        hg = np.asarray(Ws[s_id][:, :D], np.float32).T / len(sel) * scale
        hc = np.asarray(Ws[s_id][:, D:], np.float32).T * scale
        h = np.concatenate([hg, hc], axis=0)  # [1024, 512]
        h = np.clip(h, -lim, lim)
        per.append(h.reshape(8, 128, 512).transpose(1, 0, 2))
    return np.ascontiguousarray(np.stack(per, axis=1)).astype(dt)


def _kxm_e3(w_t, kchunks):
    K, M = w_t.shape
    w = np.clip(np.asarray(w_t, np.float32) * SW3, -15.5, 15.5)
    return np.ascontiguousarray(
        w.reshape(kchunks, 128, M).transpose(1, 0, 2)).astype(E3)


def _to_kxm_bf16(w_t, kchunks):
    K, M = w_t.shape
    return np.ascontiguousarray(
        np.asarray(w_t, np.float32).reshape(kchunks, 128, M)
        .transpose(1, 0, 2)).astype(BF)


def _vec_to_pm(v, chunks):
    return np.ascontiguousarray(np.asarray(v, np.float32).reshape(chunks, 128).T)


def _prep_weights(inputs):
    w = {}
    w["wa"] = _kxm_pairs(np.asarray(inputs["Wa"], np.float32).T, SW, E4)
    w["wm"] = _kxm_pairs(np.asarray(inputs["Wm"], np.float32).T, SW, E4)
    w["wq"] = _to_kxm_bf16(np.asarray(inputs["Wq"], np.float32).T, 4)
    # Wvm/W_hh halved (device tracks h2 = 2h), shipped e3m4 x 64
    w["wvm"] = _kxm_e3(np.asarray(inputs["Wvm"], np.float32).T / 2, 4)
    wih = _kxm_pairs(np.asarray(inputs["W_ih"], np.float32).T, SW, E4)
    # [128, 8, 2, 2048] -> [128, mi 16, pair 8, 2, 128]
    w["wih"] = np.ascontiguousarray(
        wih.reshape(128, 8, 2, 16, 128).transpose(0, 3, 1, 2, 4))
    w["whh"] = _kxm_e3(np.asarray(inputs["W_hh"], np.float32).T / 2, 4)
    w["w1"] = _bank_dr(np.asarray(inputs["W1"], np.float32), SELS_M,
                       S_OBJ1 / S_CONDM)
    w["w2"] = _bank_dr(np.asarray(inputs["W2"], np.float32), SELS_Q,
                       S_OBJ2 / S_QP)
    w["w2g"] = _bank_dr(np.asarray(inputs["gW2"], np.float32), SELS_Q,
                        S_OBJ2 / S_QP)
    w["w3"] = _bank_dr(np.asarray(inputs["W3"], np.float32), SELS_VM,
                       S_OBJ3 / S_VMP)
    w["w4"] = _bank_bf16(np.asarray(inputs["W4"], np.float32), SELS_VQ)
    w["w4g"] = _bank_bf16(np.asarray(inputs["gW4"], np.float32), SELS_VQ,
                          dt=E3, scale=SW3)

    bias = np.zeros((128, NBIAS), np.float32)
    bias[:, COL_LN2] = np.log(S_OBJ2)
    bias[:, COL_LN3] = np.log(S_OBJ3 / 2)
    bias[:, COL_LNH] = np.log(0.5)
    bias[:, BOFF_A:BOFF_A + 4] = _vec_to_pm(inputs["ba"], 4) * S_OBJ1
    bias[:, BOFF_M:BOFF_M + 4] = _vec_to_pm(inputs["bm"], 4) * S_CONDM
    bias[:, BOFF_Q:BOFF_Q + 4] = _vec_to_pm(inputs["bq"], 4)
    bias[:, BOFF_VM:BOFF_VM + 4] = _vec_to_pm(inputs["bvm"], 4) * S_VMP
    bias[:, BOFF_G:BOFF_G + 16] = _vec_to_pm(
        np.asarray(inputs["b_ih"], np.float32)
        + np.asarray(inputs["b_hh"], np.float32), 16)
    for si in range(len(SELS_M)):
        b = _vec_to_pm(inputs["b1"][si + 1], 4)
        bias[:, BOFF_1 + si * 4:BOFF_1 + si * 4 + 4] = b + np.log(S_OBJ2)
        bias[:, BOFF_1L + si * 4:BOFF_1L + si * 4 + 4] = b * S_OBJ2
    for si in range(len(SELS_Q)):
        b = _vec_to_pm(inputs["b2"][si + 1], 4)
        bias[:, BOFF_2 + si * 4:BOFF_2 + si * 4 + 4] = b + np.log(S_OBJ3 / 2)
        bias[:, BOFF_2L + si * 4:BOFF_2L + si * 4 + 4] = b * (S_OBJ3 / 2)
        bias[:, BOFF_G2 + si * 4:BOFF_G2 + si * 4 + 4] = _vec_to_pm(
            inputs["gb2"][si + 1], 4) / 2
    for si in range(len(SELS_VM)):
        b = _vec_to_pm(inputs["b3"][si + 1], 4)
        bias[:, BOFF_3 + si * 4:BOFF_3 + si * 4 + 4] = b
        bias[:, BOFF_3L + si * 4:BOFF_3L + si * 4 + 4] = b
    for si in range(len(SELS_VQ)):
        b = _vec_to_pm(inputs["b4"][si + 1], 4)
        bias[:, BOFF_4 + si * 4:BOFF_4 + si * 4 + 4] = b + np.log(0.5)
        bias[:, BOFF_4L + si * 4:BOFF_4L + si * 4 + 4] = b / 2
        bias[:, BOFF_G4 + si * 4:BOFF_G4 + si * 4 + 4] = _vec_to_pm(
            inputs["gb4"][si + 1], 4) / 2
    w["bias"] = bias
    return w


def _prep_core_inputs(inputs, core):
    b0 = core * BS
    app = np.asarray(inputs["appearance_video_feat"][b0:b0 + BS], np.float32)
    mot = np.asarray(inputs["motion_video_feat"][b0:b0 + BS], np.float32)
    q = np.asarray(inputs["question_embedding"][b0:b0 + BS], np.float32)
    # app [BS, C, F, V] -> [p, cc, half, pair, i, (f_h j)] e4m3 (x S_APP)
    app_t = app.transpose(3, 2, 0, 1).reshape(V, F, J)          # [V, F, J]
    app_t = app_t.reshape(8, 2, 128, F, J).transpose(2, 0, 1, 3, 4)  # [p,pr,i,F,J]
    app_t = app_t.reshape(128, 8, 2, 4, 2, 2, J)                # F -> cc,h,f_h
    app_t = app_t.transpose(0, 3, 4, 1, 2, 5, 6).reshape(128, 4, 2, 8, 2, 256)
    app_t = np.clip(app_t * S_APP, -240, 240)
    # mot [BS, C, V] -> [p, pair, i, j] e4m3 (x S_MOT)
    mot_t = mot.transpose(2, 0, 1).reshape(V, J)
    mot_t = mot_t.reshape(8, 2, 128, J).transpose(2, 0, 1, 3)
    mot_t = np.clip(mot_t * S_MOT, -240, 240)
    q_t = q.T.reshape(4, 128, BS).transpose(1, 0, 2)
    return {
        "app": np.ascontiguousarray(app_t).astype(E4),
        "mot": np.ascontiguousarray(mot_t).astype(E4),
        "q": np.ascontiguousarray(q_t).astype(BF),
    }


def _assemble(results):
    out = np.empty((B, (C - 4) * T, D), np.float32)
    for core in range(NCORES):
        r = np.asarray(results[core]["out"], np.float32).reshape(128, 4, 4, BS, T)
        # [p, si, m, b, t] -> [b, si, t, m, p]
        o = r.transpose(3, 1, 4, 2, 0).reshape(BS, (C - 4) * T, D)
        out[core * BS:(core + 1) * BS] = o
    return out


def build_in_maps(**inputs):
    w = _prep_weights(inputs)
    in_maps = []
    for core in range(NCORES):
        m = dict(w)
        m.update(_prep_core_inputs(inputs, core))
        in_maps.append(m)
    return in_maps


def _all_biases_zero(inputs):
    names = ["ba", "bm", "bq", "bvm", "b_ih", "b_hh", "b1", "b2", "gb2",
             "b3", "b4", "gb4"]
    return all(not np.any(np.asarray(inputs[n], np.float32)) for n in names)


def kernel(**inputs):
    nc = _program(_all_biases_zero(inputs))
    in_maps = build_in_maps(**inputs)
    res = run_bass_kernel_spmd(nc, in_maps, list(range(NCORES)))
    return _assemble(res.results)


if __name__ == "__main__":
    import reference

    inputs = {k: np.asarray(v) for k, v in reference.setup_inputs().items()}
    out = kernel(**inputs)
    exp = np.asarray(reference.reference(**inputs))
    err = np.abs(out - exp).max() / np.abs(exp).max()
    print("Relative error:", err)
